# revision 25
# baseline (speedup 1.0000x reference)
"""Multi-head attention (B=8, N=1024, D=768, H=12) on 8 TRN2 NeuronCores.

Sharding: pure data parallel over batch — each core handles one batch
element; weights are replicated. No collectives.

v2 — dense-PE redesign (baseline was 296us, PE idle ~30% + pstate
resets after every stall):
  * fp16 operands everywhere on the PE (host-cast x/w_qkv/w_proj):
    same 1 col/cycle as f32r but half the DMA bytes (6.3MB vs 12.6MB),
    so the prologue and round-0 v-projection are no longer DMA-starved.
    (fp8 was measured in numpy: 4.4e-2 rel err — over the 2e-2 budget.)
  * qk bias folded into the DVE PSUM->SBUF cast (tensor_scalar with a
    per-partition [128,1] bias column) — kills 24 PE bias matmuls.
  * PSUM split into two fixed 2x[128,1024] pools (16KB/part total):
    A: scores halves (double-buffered at qh granularity, so the exp
       of half k never blocks the scores matmul of half k+1), the
       round-tail qk^T prefetch burst, and the epilogue proj tiles.
    B: round-0 v tiles, then attn@v accumulators (i=0/1), then shared
       with proj in the final round.
  * attn@v PSUM released by ONE DVE copy ([65,1024] -> SBUF) right
    after the last accumulation; the softmax normalization (gpsimd
    partition_broadcast of the den row + DVE reciprocal + multiply)
    runs entirely off the PE critical path.  No DRAM bounce.
  * exp per [128,1024] qh-half (96 ACT instrs, ~983ns each) paced
    against per-kt PE work; scores/attn@v/prefetch interleaved so the
    Tensor engine never idles -> stays at the 2.4GHz pstate instead of
    dropping to 1.2GHz after each stall.
"""

import sys

sys.path.insert(0, "/opt/trn_rl_repo")

import numpy as np

B, N, D, H, HD = 8, 1024, 768, 12, 64
F_QK = 2 * D  # 1536
SCALE = HD**-0.5
TOK_TILES = N // 128  # 8
D_SUB = D // 128  # 6
N_CORES = 8

_cached_nc = None


def _build():
    import concourse.tile as tile
    from concourse import bacc, mybir

    F32 = mybir.dt.float32
    FP16 = mybir.dt.float16
    EXP = mybir.ActivationFunctionType.Exp
    MULT = mybir.AluOpType.mult
    ADD = mybir.AluOpType.add

    nc = bacc.Bacc("TRN2", target_bir_lowering=False, debug=False)

    xt_d = nc.dram_tensor("xt", [D, N], FP16, kind="ExternalInput").ap()
    wqk_d = nc.dram_tensor("wqk", [D, F_QK], FP16, kind="ExternalInput").ap()
    wv_d = nc.dram_tensor("wv", [D, D], FP16, kind="ExternalInput").ap()
    wproj_d = nc.dram_tensor("wproj", [D, D], FP16, kind="ExternalInput").ap()
    bqk_d = nc.dram_tensor("bqk", [128, 12], F32, kind="ExternalInput").ap()
    bv_d = nc.dram_tensor("bv", [D], FP16, kind="ExternalInput").ap()
    bp_d = nc.dram_tensor("bp", [D], FP16, kind="ExternalInput").ap()
    y_d = nc.dram_tensor("y", [N, D], FP16, kind="ExternalOutput").ap()

    with tile.TileContext(nc) as tc:
        with (
            tc.tile_pool(name="singles", bufs=1) as singles,
            tc.tile_pool(name="qkT", bufs=7) as qkT_pool,
            tc.tile_pool(name="wqk", bufs=16) as wqk_pool,
            tc.tile_pool(name="attnT", bufs=12) as attnT_pool,
            tc.tile_pool(name="aoraw", bufs=4) as aoraw_pool,
            tc.tile_pool(name="den", bufs=4) as den_pool,
            tc.tile_pool(name="yout", bufs=3) as y_pool,
            tc.tile_pool(name="psA", bufs=2, space="PSUM") as psA,
            tc.tile_pool(name="psB", bufs=2, space="PSUM") as psB,
            tc.tile_pool(name="dram", bufs=2, space="DRAM") as dram_pool,
        ):
            # ---- resident SBUF tensors ----
            xT_sb = singles.tile([128, D_SUB, N], FP16)  # 12KB/part
            v_sb = singles.tile([128, TOK_TILES, H * 65], FP16)  # 12.2KB/part
            aoT_sb = singles.tile([128, D_SUB, N], FP16)  # 12KB/part
            wproj_sb = singles.tile([128, D_SUB, D], FP16)  # 9KB/part
            wv_sb = singles.tile([128, D_SUB, D], FP16)  # 9KB/part
            bqk_sb = singles.tile([128, 12], F32)
            bv_sb = singles.tile([1, D], FP16)
            bp_sb = singles.tile([1, D], FP16)
            ones1 = singles.tile([1, 512], FP16)
            ones16 = singles.tile([128, 96], FP16)
            ones_f = singles.tile([128, 512], F32)

            # ---- prologue DMAs, interleaved for earliest PE start ----
            # critical chain: wt(f,d) + x half (d, qh0) feed the first
            # qk^T chains; v-weights follow; wproj trails (needed last).
            xt_r = xt_d.rearrange("(o p) n -> p o n", p=128)

            def dma_wt(eng, f, d):
                wt = wqk_pool.tile([128, 128], FP16, tag="wqk", name=f"wt_{f}_{d}")
                eng.dma_start(wt, wqk_d[d * 128 : (d + 1) * 128, f * 128 : f * 128 + 128])
                return wt

            # three issue queues in parallel: sync=x halves + first wqk
            # tiles (critical chain, interleaved so the f0-qh0 matmuls can
            # start on the first arrivals), gpsimd=biases, scalar=bulk.
            wt_tiles = {}
            nc.gpsimd.dma_start(bqk_sb, bqk_d)
            for d in range(D_SUB):
                wt_tiles[(0, d)] = dma_wt(nc.sync, 0, d)
                nc.sync.dma_start(xT_sb[:, d, 0:512], xt_r[:, d, 0:512])
            for d in range(D_SUB):
                wt_tiles[(6, d)] = dma_wt(nc.sync, 6, d)
            for d in range(D_SUB):
                nc.sync.dma_start(xT_sb[:, d, 512:N], xt_r[:, d, 512:N])
            nc.scalar.dma_start(wv_sb, wv_d.rearrange("(o p) f -> p o f", p=128))
            nc.gpsimd.dma_start(bv_sb, bv_d[None, :])
            nc.scalar.dma_start(wproj_sb, wproj_d.rearrange("(o p) f -> p o f", p=128))
            nc.gpsimd.dma_start(bp_sb, bp_d[None, :])

            nc.vector.memset(ones_f, 1.0)
            nc.vector.tensor_copy(ones1, ones_f[0:1, :])
            nc.vector.tensor_copy(ones16, ones_f[:, 0:96])
            # ones columns of [v | 1] slots
            v_ones_view = v_sb.rearrange("p s (h c) -> p s h c", c=65)[:, :, :, 64]
            nc.vector.tensor_copy(
                v_ones_view, ones16.rearrange("p (s h) -> p s h", s=8)
            )

            qk_tiles = {}

            # ---- qk^T: one 128-feature tile (f in 0..11), fp16 out ----
            # bias folded into the PSUM->SBUF cast (per-partition add).
            # matmuls and cast are separately emittable so round tails can
            # order the DVE queue as [drains, casts] — the drains gate the
            # next round's attn@v.
            qk_psq = {}
            qk_cast_left = {}

            def emit_qk_matmuls(f, wt_eng, qhs=(0, 1), pool=None):
                if (f, 0) not in wt_tiles:
                    for d in range(D_SUB):
                        wt_tiles[(f, d)] = dma_wt(wt_eng, f, d)
                if f not in qk_psq:
                    pool = pool or psA
                    tg = "psA" if pool is psA else "psB"
                    qk_psq[f] = pool.tile([128, N], F32, tag=tg, name=f"psq_{f}")
                    qk_cast_left[f] = 2
                psq = qk_psq[f]
                for qh in qhs:
                    sl = slice(qh * 512, (qh + 1) * 512)
                    for d in range(D_SUB):
                        nc.tensor.matmul(
                            psq[:, sl],
                            lhsT=wt_tiles[(f, d)],
                            rhs=xT_sb[:, d, sl],
                            start=(d == 0),
                            stop=(d == D_SUB - 1),
                        )

            def emit_qk_cast(f, qhs=(0, 1)):
                # per-qh-half casts: a half only needs the x halves that fed
                # it, so the first scores/exps can start before all of x has
                # landed
                psq = qk_psq[f]
                if f not in qk_tiles:
                    qk_tiles[f] = qkT_pool.tile(
                        [128, N], FP16, tag="qkT", name=f"qkT_{f}"
                    )
                qt = qk_tiles[f]
                with tc.high_priority():
                    for qh in qhs:
                        sl = slice(qh * 512, (qh + 1) * 512)
                        nc.vector.tensor_scalar(
                            out=qt[:, sl], in0=psq[:, sl],
                            scalar1=bqk_sb[:, f : f + 1], scalar2=None, op0=ADD,
                        )
                        qk_cast_left[f] -= 1
                if qk_cast_left[f] == 0:
                    del qk_psq[f]

            def emit_qk_tile(f, wt_eng, pool=None):
                emit_qk_matmuls(f, wt_eng, pool=pool)
                emit_qk_cast(f)

            # ---- v m-tile: natural layout, scattered into 65-slots ----
            def emit_v_tile(m):
                psv = psB.tile([128, N], F32, tag="psB", name=f"psv_{m}")
                for n0, nsz in ((0, 512), (512, 256)):
                    sl = slice(n0, n0 + nsz)
                    for d in range(D_SUB):
                        nc.tensor.matmul(
                            psv[:, sl],
                            lhsT=xT_sb[:, d, m * 128 : (m + 1) * 128],
                            rhs=wv_sb[:, d, sl],
                            start=(d == 0),
                            stop=False,
                        )
                    nc.tensor.matmul(
                        psv[:, sl],
                        lhsT=ones1[0:1, 0:128],
                        rhs=bv_sb[0:1, sl],
                        start=False,
                        stop=True,
                    )
                nc.vector.tensor_copy(
                    v_sb[:, m, :].rearrange("p (h c) -> p h c", c=65)[:, :, 0:64],
                    psv[:, 0:D].rearrange("p (h c) -> p h c", c=64),
                )

            # ---- attention rounds, software-pipelined over head pairs ----
            attn_tiles = {}  # (pair, kt) -> [128, 2048] fp16: [A0|B0|A1|B1]
            pso_live = {}

            def emit_scores_half(p, kt, qh):
                # the exp stream on ACT is the near-critical path: keep the
                # whole scores->exp chain at the front of the scheduler's
                # priority heap so it is never deferred behind bulk matmuls
                qT = qk_tiles[p]
                kT = qk_tiles[6 + p]
                pss = psA.tile([128, N], F32, tag="psA", name=f"pss_{p}_{kt}_{qh}")
                with tc.high_priority():
                    for i in range(2):
                        pb = slice(64 * i, 64 * i + 64)
                        nc.tensor.matmul(
                            pss[:, i * 512 : i * 512 + 512],
                            lhsT=kT[pb, kt * 128 : (kt + 1) * 128],
                            rhs=qT[pb, qh * 512 : (qh + 1) * 512],
                            start=True,
                            stop=True,
                        )
                    at = attn_tiles[(p, kt)]
                    nc.scalar.activation(
                        at[:, qh * 1024 : (qh + 1) * 1024], pss, func=EXP, scale=SCALE
                    )

            def emit_scores_kt(p, kt, qhs=(0, 1)):
                if (p, kt) not in attn_tiles:
                    attn_tiles[(p, kt)] = attnT_pool.tile(
                        [128, 2 * N], FP16, tag="attnT", name=f"at_{p}_{kt}"
                    )
                for qh in qhs:
                    emit_scores_half(p, kt, qh)

            def emit_attnv_kt(p, kt):
                at = attn_tiles[(p, kt)]
                for i in range(2):
                    h = 2 * p + i
                    for qh in range(2):
                        osl = slice(qh * 512, (qh + 1) * 512)
                        isl = slice(qh * 1024 + i * 512, qh * 1024 + i * 512 + 512)
                        nc.tensor.matmul(
                            pso_live[i][0:65, osl],
                            lhsT=v_sb[:, kt, h * 65 : h * 65 + 65],
                            rhs=at[:, isl],
                            start=(kt == 0),
                            stop=(kt == TOK_TILES - 1),
                        )
                if kt == TOK_TILES - 1:
                    attn_tiles.pop((p, kt))
                else:
                    del attn_tiles[(p, kt)]

            def emit_drain(p, i):
                # single DVE copy releases the PSUM accumulator fast; high
                # priority so it isn't queued behind other DVE work (it
                # gates the next round's attn@v accumulators)
                h = 2 * p + i
                raw = aoraw_pool.tile([65, N], F32, tag="aoraw", name=f"raw_{h}")
                with tc.high_priority():
                    nc.vector.tensor_copy(raw, pso_live[i][0:65, :])
                return raw

            def emit_norm(p, i, raw):
                # den row -> DRAM-bounce broadcast to 64 partitions
                # (partition-step-0 read is legal from DRAM; the gpsimd
                # partition_broadcast reads physical partition 0 on HW) ->
                # reciprocal -> multiply into fp16 aoT
                import concourse.bass as bass

                h = 2 * p + i
                dend = dram_pool.tile([1, N], F32, tag="dend", name=f"dend_{h}")
                nc.sync.dma_start(dend, raw[64:65, :])
                denb = den_pool.tile([64, N], F32, tag="den", name=f"denb_{h}")
                dend_bcast = bass.AP(
                    tensor=dend.tensor,
                    offset=dend.offset,
                    ap=[[0, 64]] + list(dend.ap[1:]),
                )
                nc.sync.dma_start(denb, dend_bcast)
                denr = den_pool.tile([64, N], F32, tag="den", name=f"denr_{h}")
                nc.vector.reciprocal_approx_fast(out=denr, in_=denb)
                nc.vector.tensor_tensor(
                    aoT_sb[64 * i : 64 * i + 64, p, :],
                    raw[0:64, :],
                    denr,
                    MULT,
                )

            # proj helpers (epilogue, pairwise m-tiles in pool A)
            def proj_partial(psy, m, d_range, with_bias):
                for n0, nsz in ((0, 512), (512, 256)):
                    sl = slice(n0, n0 + nsz)
                    for d in d_range:
                        nc.tensor.matmul(
                            psy[:, sl],
                            lhsT=aoT_sb[:, d, m * 128 : (m + 1) * 128],
                            rhs=wproj_sb[:, d, sl],
                            start=(d == 0),
                            stop=False,
                        )
                    if with_bias:
                        nc.tensor.matmul(
                            psy[:, sl],
                            lhsT=ones1[0:1, 0:128],
                            rhs=bp_sb[0:1, sl],
                            start=False,
                            stop=True,
                        )

            def proj_finish(psy, m):
                ysb = y_pool.tile([128, D], FP16, tag="ysb", name=f"ysb_{m}")
                nc.vector.tensor_copy(ysb, psy[:, 0:D])
                nc.sync.dma_start(y_d[m * 128 : (m + 1) * 128, :], ysb)

            # ---- prologue: pair-0 qk tiles half-by-half so the first
            # scores/exps start as soon as the qh0 x-halves land (ACT's
            # ~110us exp stream is the near-critical path).  The prologue
            # psq accumulators live in pool B so the scores' pss rotation
            # in pool A is never blocked behind a late cast. ----
            emit_qk_matmuls(0, nc.sync, qhs=(0,), pool=psB)
            emit_qk_matmuls(6, nc.sync, qhs=(0,), pool=psB)
            emit_qk_cast(0, qhs=(0,))
            emit_qk_cast(6, qhs=(0,))
            emit_scores_kt(0, 0, qhs=(0,))
            emit_scores_kt(0, 1, qhs=(0,))
            emit_qk_matmuls(0, nc.sync, qhs=(1,))
            emit_qk_matmuls(6, nc.sync, qhs=(1,))
            emit_qk_cast(0, qhs=(1,))
            emit_qk_cast(6, qhs=(1,))
            emit_scores_kt(0, 0, qhs=(1,))
            emit_scores_kt(0, 1, qhs=(1,))
            emit_qk_tile(1, nc.gpsimd, pool=psB)  # q heads 2,3
            emit_v_tile(0)
            emit_qk_tile(7, nc.gpsimd, pool=psB)  # k heads 2,3
            emit_v_tile(1)

            # round 0: remaining v tiles + scores pair 0 (skewed +2)
            for j in range(2, TOK_TILES):
                emit_v_tile(j)
                emit_scores_kt(0, j)
            # r0 tail: next round's first scores BEFORE the prefetch burst
            # (their pss slots must not rotate behind the psq tiles)
            emit_scores_kt(1, 0)
            emit_scores_kt(1, 1)
            emit_qk_matmuls(2, nc.gpsimd)
            emit_qk_matmuls(8, nc.gpsimd)
            emit_qk_cast(2)
            emit_qk_cast(8)

            # rounds 1..5: attn@v pair r-1, scores pair r (skewed +2)
            for r in range(1, 6):
                pso_live = {
                    i: psB.tile([128, N], F32, tag="psB", name=f"pso_{r - 1}_{i}")
                    for i in range(2)
                }
                for kt in range(TOK_TILES):
                    emit_attnv_kt(r - 1, kt)
                    if kt < 6:
                        emit_scores_kt(r, kt + 2)
                # tail: next round's first scores IMMEDIATELY (pss slots
                # free as the last exps retire; their qk tiles were cast a
                # round ago) so ACT never idles across the boundary; then
                # the prefetch bursts; DVE order = drain0, castA, castB,
                # drain1 so both attn@v accumulators and the kt-loop's
                # first cast-dependent scores unblock just in time.
                if r < 5:
                    emit_scores_kt(r + 1, 0)
                    emit_scores_kt(r + 1, 1)
                raw0 = emit_drain(r - 1, 0)
                if r + 2 < 6:
                    emit_qk_matmuls(r + 2, nc.gpsimd)
                    emit_qk_cast(r + 2)
                    emit_qk_matmuls(6 + r + 2, nc.gpsimd)
                    emit_qk_cast(6 + r + 2)
                raw1 = emit_drain(r - 1, 1)
                emit_norm(r - 1, 0, raw0)
                emit_norm(r - 1, 1, raw1)

            # round 6: attn@v pair 5 + first proj partials
            pso_live = {
                i: psB.tile([128, N], F32, tag="psB", name=f"pso_5_{i}")
                for i in range(2)
            }
            psy0 = psA.tile([128, N], F32, tag="psA", name="psy_0")
            psy1 = psA.tile([128, N], F32, tag="psA", name="psy_1")
            proj_partial(psy0, 0, range(5), False)
            for kt in range(TOK_TILES):
                emit_attnv_kt(5, kt)
            raw0 = emit_drain(5, 0)
            raw1 = emit_drain(5, 1)
            proj_partial(psy1, 1, range(5), False)
            # pool B is free after the drains: fill the norm-5 latency
            # with proj partials for m=2,3 there
            psy2 = psB.tile([128, N], F32, tag="psB", name="psy_2")
            proj_partial(psy2, 2, range(5), False)
            psy3 = psB.tile([128, N], F32, tag="psB", name="psy_3")
            proj_partial(psy3, 3, range(5), False)
            emit_norm(5, 0, raw0)
            emit_norm(5, 1, raw1)

            # ---- output projection (m=0..3 partials already queued) ----
            for psy, m in ((psy0, 0), (psy1, 1), (psy2, 2), (psy3, 3)):
                proj_partial(psy, m, range(5, D_SUB), True)
            for psy, m in ((psy0, 0), (psy1, 1), (psy2, 2), (psy3, 3)):
                proj_finish(psy, m)
            for m, pool, tg in ((4, psA, "psA"), (5, psA, "psA"), (6, psB, "psB"), (7, psB, "psB")):
                psy = pool.tile([128, N], F32, tag=tg, name=f"psy_{m}")
                proj_partial(psy, m, range(D_SUB), True)
                proj_finish(psy, m)

    nc.compile()
    return nc


def _in_maps(x, w_qkv, b_qkv, w_proj, b_proj):
    w_qkv = np.asarray(w_qkv, dtype=np.float32)
    b_qkv = np.asarray(b_qkv, dtype=np.float32)
    w_proj = np.asarray(w_proj, dtype=np.float32)
    b_proj = np.asarray(b_proj, dtype=np.float32)
    wqk16 = np.ascontiguousarray(w_qkv[:, :F_QK], dtype=np.float16)
    wv16 = np.ascontiguousarray(w_qkv[:, F_QK:], dtype=np.float16)
    wp16 = np.ascontiguousarray(w_proj, dtype=np.float16)
    bqk_col = np.ascontiguousarray(
        b_qkv[:F_QK].reshape(12, 128).T, dtype=np.float32
    )
    bv16 = np.ascontiguousarray(b_qkv[F_QK:], dtype=np.float16)
    bp16 = np.ascontiguousarray(b_proj, dtype=np.float16)
    maps = []
    for c in range(N_CORES):
        maps.append(
            {
                "xt": np.ascontiguousarray(
                    np.asarray(x[c], dtype=np.float32).T.astype(np.float16)
                ),
                "wqk": wqk16,
                "wv": wv16,
                "wproj": wp16,
                "bqk": bqk_col,
                "bv": bv16,
                "bp": bp16,
            }
        )
    return maps


def kernel(x, w_qkv, b_qkv, w_proj, b_proj):
    global _cached_nc
    if _cached_nc is None:
        _cached_nc = _build()
    from concourse.bass_utils import run_bass_kernel_spmd

    res = run_bass_kernel_spmd(
        _cached_nc,
        _in_maps(x, w_qkv, b_qkv, w_proj, b_proj),
        list(range(N_CORES)),
    )
    return np.stack(
        [res.results[c]["y"].astype(np.float32) for c in range(N_CORES)]
    )


if __name__ == "__main__":
    rng = np.random.default_rng(0)
    x = rng.standard_normal((B, N, D), dtype=np.float32)
    w_qkv = rng.standard_normal((D, 3 * D), dtype=np.float32) * D**-0.5
    b_qkv = rng.standard_normal(3 * D).astype(np.float32) * 0.01
    w_proj = rng.standard_normal((D, D), dtype=np.float32) * D**-0.5
    b_proj = rng.standard_normal(D).astype(np.float32) * 0.01
    y = kernel(x, w_qkv, b_qkv, w_proj, b_proj)
    print(y.shape, y.dtype)


# revision 29
# speedup vs baseline: 1.0080x; 1.0080x over previous
"""Multi-head attention (B=8, N=1024, D=768, H=12) on 8 TRN2 NeuronCores.

Sharding: pure data parallel over batch — each core handles one batch
element; weights are replicated. No collectives.

v2 — dense-PE redesign (baseline was 296us, PE idle ~30% + pstate
resets after every stall):
  * fp16 operands everywhere on the PE (host-cast x/w_qkv/w_proj):
    same 1 col/cycle as f32r but half the DMA bytes (6.3MB vs 12.6MB),
    so the prologue and round-0 v-projection are no longer DMA-starved.
    (fp8 was measured in numpy: 4.4e-2 rel err — over the 2e-2 budget.)
  * qk bias folded into the DVE PSUM->SBUF cast (tensor_scalar with a
    per-partition [128,1] bias column) — kills 24 PE bias matmuls.
  * PSUM split into two fixed 2x[128,1024] pools (16KB/part total):
    A: scores halves (double-buffered at qh granularity, so the exp
       of half k never blocks the scores matmul of half k+1), the
       round-tail qk^T prefetch burst, and the epilogue proj tiles.
    B: round-0 v tiles, then attn@v accumulators (i=0/1), then shared
       with proj in the final round.
  * attn@v PSUM released by ONE DVE copy ([65,1024] -> SBUF) right
    after the last accumulation; the softmax normalization (gpsimd
    partition_broadcast of the den row + DVE reciprocal + multiply)
    runs entirely off the PE critical path.  No DRAM bounce.
  * exp per [128,1024] qh-half (96 ACT instrs, ~983ns each) paced
    against per-kt PE work; scores/attn@v/prefetch interleaved so the
    Tensor engine never idles -> stays at the 2.4GHz pstate instead of
    dropping to 1.2GHz after each stall.
"""

import sys

sys.path.insert(0, "/opt/trn_rl_repo")

import numpy as np

B, N, D, H, HD = 8, 1024, 768, 12, 64
F_QK = 2 * D  # 1536
SCALE = HD**-0.5
TOK_TILES = N // 128  # 8
D_SUB = D // 128  # 6
N_CORES = 8

_cached_nc = None


def _build():
    import concourse.tile as tile
    from concourse import bacc, mybir

    F32 = mybir.dt.float32
    FP16 = mybir.dt.float16
    EXP = mybir.ActivationFunctionType.Exp
    MULT = mybir.AluOpType.mult
    ADD = mybir.AluOpType.add

    nc = bacc.Bacc("TRN2", target_bir_lowering=False, debug=False)

    xt_d = nc.dram_tensor("xt", [D, N], FP16, kind="ExternalInput").ap()
    wqk_d = nc.dram_tensor("wqk", [D, F_QK], FP16, kind="ExternalInput").ap()
    wv_d = nc.dram_tensor("wv", [D, D], FP16, kind="ExternalInput").ap()
    wproj_d = nc.dram_tensor("wproj", [D, D], FP16, kind="ExternalInput").ap()
    bqk_d = nc.dram_tensor("bqk", [128, 12], F32, kind="ExternalInput").ap()
    bv_d = nc.dram_tensor("bv", [D], FP16, kind="ExternalInput").ap()
    bp_d = nc.dram_tensor("bp", [D], FP16, kind="ExternalInput").ap()
    y_d = nc.dram_tensor("y", [N, D], FP16, kind="ExternalOutput").ap()

    with tile.TileContext(nc) as tc:
        with (
            tc.tile_pool(name="singles", bufs=1) as singles,
            tc.tile_pool(name="qkT", bufs=7) as qkT_pool,
            tc.tile_pool(name="wqk", bufs=16) as wqk_pool,
            tc.tile_pool(name="attnT", bufs=12) as attnT_pool,
            tc.tile_pool(name="aoraw", bufs=4) as aoraw_pool,
            tc.tile_pool(name="den", bufs=4) as den_pool,
            tc.tile_pool(name="yout", bufs=3) as y_pool,
            tc.tile_pool(name="psA", bufs=2, space="PSUM") as psA,
            tc.tile_pool(name="psB", bufs=2, space="PSUM") as psB,
            tc.tile_pool(name="dram", bufs=2, space="DRAM") as dram_pool,
        ):
            # ---- resident SBUF tensors ----
            xT_sb = singles.tile([128, D_SUB, N], FP16)  # 12KB/part
            v_sb = singles.tile([128, TOK_TILES, H * 65], FP16)  # 12.2KB/part
            aoT_sb = singles.tile([128, D_SUB, N], FP16)  # 12KB/part
            wproj_sb = singles.tile([128, D_SUB, D], FP16)  # 9KB/part
            wv_sb = singles.tile([128, D_SUB, D], FP16)  # 9KB/part
            bqk_sb = singles.tile([128, 12], F32)
            bv_sb = singles.tile([1, D], FP16)
            bp_sb = singles.tile([1, D], FP16)
            ones1 = singles.tile([1, 512], FP16)
            ones16 = singles.tile([128, 96], FP16)
            ones_f = singles.tile([128, 512], F32)

            # ---- prologue DMAs, interleaved for earliest PE start ----
            # critical chain: wt(f,d) + x half (d, qh0) feed the first
            # qk^T chains; v-weights follow; wproj trails (needed last).
            xt_r = xt_d.rearrange("(o p) n -> p o n", p=128)

            def dma_wt(eng, f, d):
                wt = wqk_pool.tile([128, 128], FP16, tag="wqk", name=f"wt_{f}_{d}")
                eng.dma_start(wt, wqk_d[d * 128 : (d + 1) * 128, f * 128 : f * 128 + 128])
                return wt

            # three issue queues in parallel: sync=x halves + first wqk
            # tiles (critical chain, interleaved so the f0-qh0 matmuls can
            # start on the first arrivals), gpsimd=biases, scalar=bulk.
            wt_tiles = {}
            nc.gpsimd.dma_start(bqk_sb, bqk_d)
            for d in range(D_SUB):
                wt_tiles[(0, d)] = dma_wt(nc.sync, 0, d)
                nc.sync.dma_start(xT_sb[:, d, 0:512], xt_r[:, d, 0:512])
            # second half of the critical inputs on the scalar queue: the
            # per-queue descriptor issue rate (~0.65us each) was the
            # prologue floor with everything on sync
            for d in range(D_SUB):
                wt_tiles[(6, d)] = dma_wt(nc.scalar, 6, d)
                nc.scalar.dma_start(xT_sb[:, d, 512:N], xt_r[:, d, 512:N])
            nc.scalar.dma_start(wv_sb, wv_d.rearrange("(o p) f -> p o f", p=128))
            nc.gpsimd.dma_start(bv_sb, bv_d[None, :])
            nc.sync.dma_start(wproj_sb, wproj_d.rearrange("(o p) f -> p o f", p=128))
            nc.gpsimd.dma_start(bp_sb, bp_d[None, :])

            nc.vector.memset(ones_f, 1.0)
            nc.vector.tensor_copy(ones1, ones_f[0:1, :])
            nc.vector.tensor_copy(ones16, ones_f[:, 0:96])
            # ones columns of [v | 1] slots
            v_ones_view = v_sb.rearrange("p s (h c) -> p s h c", c=65)[:, :, :, 64]
            nc.vector.tensor_copy(
                v_ones_view, ones16.rearrange("p (s h) -> p s h", s=8)
            )

            qk_tiles = {}

            # ---- qk^T: one 128-feature tile (f in 0..11), fp16 out ----
            # bias folded into the PSUM->SBUF cast (per-partition add).
            # matmuls and cast are separately emittable so round tails can
            # order the DVE queue as [drains, casts] — the drains gate the
            # next round's attn@v.
            qk_psq = {}
            qk_cast_left = {}

            def emit_qk_matmuls(f, wt_eng, qhs=(0, 1), pool=None):
                if (f, 0) not in wt_tiles:
                    for d in range(D_SUB):
                        wt_tiles[(f, d)] = dma_wt(wt_eng, f, d)
                if f not in qk_psq:
                    pool = pool or psA
                    tg = "psA" if pool is psA else "psB"
                    qk_psq[f] = pool.tile([128, N], F32, tag=tg, name=f"psq_{f}")
                    qk_cast_left[f] = 2
                psq = qk_psq[f]
                for qh in qhs:
                    sl = slice(qh * 512, (qh + 1) * 512)
                    for d in range(D_SUB):
                        nc.tensor.matmul(
                            psq[:, sl],
                            lhsT=wt_tiles[(f, d)],
                            rhs=xT_sb[:, d, sl],
                            start=(d == 0),
                            stop=(d == D_SUB - 1),
                        )

            def emit_qk_cast(f, qhs=(0, 1)):
                # per-qh-half casts: a half only needs the x halves that fed
                # it, so the first scores/exps can start before all of x has
                # landed
                psq = qk_psq[f]
                if f not in qk_tiles:
                    qk_tiles[f] = qkT_pool.tile(
                        [128, N], FP16, tag="qkT", name=f"qkT_{f}"
                    )
                qt = qk_tiles[f]
                with tc.high_priority():
                    for qh in qhs:
                        sl = slice(qh * 512, (qh + 1) * 512)
                        nc.vector.tensor_scalar(
                            out=qt[:, sl], in0=psq[:, sl],
                            scalar1=bqk_sb[:, f : f + 1], scalar2=None, op0=ADD,
                        )
                        qk_cast_left[f] -= 1
                if qk_cast_left[f] == 0:
                    del qk_psq[f]

            def emit_qk_tile(f, wt_eng, pool=None):
                emit_qk_matmuls(f, wt_eng, pool=pool)
                emit_qk_cast(f)

            # ---- v m-tile: natural layout, scattered into 65-slots ----
            def emit_v_tile(m):
                psv = psB.tile([128, N], F32, tag="psB", name=f"psv_{m}")
                for n0, nsz in ((0, 512), (512, 256)):
                    sl = slice(n0, n0 + nsz)
                    for d in range(D_SUB):
                        nc.tensor.matmul(
                            psv[:, sl],
                            lhsT=xT_sb[:, d, m * 128 : (m + 1) * 128],
                            rhs=wv_sb[:, d, sl],
                            start=(d == 0),
                            stop=False,
                        )
                    nc.tensor.matmul(
                        psv[:, sl],
                        lhsT=ones1[0:1, 0:128],
                        rhs=bv_sb[0:1, sl],
                        start=False,
                        stop=True,
                    )
                nc.vector.tensor_copy(
                    v_sb[:, m, :].rearrange("p (h c) -> p h c", c=65)[:, :, 0:64],
                    psv[:, 0:D].rearrange("p (h c) -> p h c", c=64),
                )

            # ---- attention rounds, software-pipelined over head pairs ----
            attn_tiles = {}  # (pair, kt) -> [128, 2048] fp16: [A0|B0|A1|B1]
            pso_live = {}

            def emit_scores_half(p, kt, qh):
                # the exp stream on ACT is the near-critical path: keep the
                # whole scores->exp chain at the front of the scheduler's
                # priority heap so it is never deferred behind bulk matmuls
                qT = qk_tiles[p]
                kT = qk_tiles[6 + p]
                pss = psA.tile([128, N], F32, tag="psA", name=f"pss_{p}_{kt}_{qh}")
                with tc.high_priority():
                    for i in range(2):
                        pb = slice(64 * i, 64 * i + 64)
                        nc.tensor.matmul(
                            pss[:, i * 512 : i * 512 + 512],
                            lhsT=kT[pb, kt * 128 : (kt + 1) * 128],
                            rhs=qT[pb, qh * 512 : (qh + 1) * 512],
                            start=True,
                            stop=True,
                        )
                    at = attn_tiles[(p, kt)]
                    nc.scalar.activation(
                        at[:, qh * 1024 : (qh + 1) * 1024], pss, func=EXP, scale=SCALE
                    )

            def emit_scores_kt(p, kt, qhs=(0, 1)):
                if (p, kt) not in attn_tiles:
                    attn_tiles[(p, kt)] = attnT_pool.tile(
                        [128, 2 * N], FP16, tag="attnT", name=f"at_{p}_{kt}"
                    )
                for qh in qhs:
                    emit_scores_half(p, kt, qh)

            def emit_attnv_kt(p, kt):
                at = attn_tiles[(p, kt)]
                for i in range(2):
                    h = 2 * p + i
                    for qh in range(2):
                        osl = slice(qh * 512, (qh + 1) * 512)
                        isl = slice(qh * 1024 + i * 512, qh * 1024 + i * 512 + 512)
                        nc.tensor.matmul(
                            pso_live[i][0:65, osl],
                            lhsT=v_sb[:, kt, h * 65 : h * 65 + 65],
                            rhs=at[:, isl],
                            start=(kt == 0),
                            stop=(kt == TOK_TILES - 1),
                        )
                if kt == TOK_TILES - 1:
                    attn_tiles.pop((p, kt))
                else:
                    del attn_tiles[(p, kt)]

            def emit_drain(p, i):
                # single DVE copy releases the PSUM accumulator fast; high
                # priority so it isn't queued behind other DVE work (it
                # gates the next round's attn@v accumulators)
                h = 2 * p + i
                raw = aoraw_pool.tile([65, N], F32, tag="aoraw", name=f"raw_{h}")
                with tc.high_priority():
                    nc.vector.tensor_copy(raw, pso_live[i][0:65, :])
                return raw

            def emit_norm_bounce(p, i, raw):
                # den row -> DRAM-bounce broadcast to 64 partitions
                # (partition-step-0 read is legal from DRAM; the gpsimd
                # partition_broadcast reads physical partition 0 on HW)
                import concourse.bass as bass

                h = 2 * p + i
                dend = dram_pool.tile([1, N], F32, tag="dend", name=f"dend_{h}")
                nc.sync.dma_start(dend, raw[64:65, :])
                denb = den_pool.tile([64, N], F32, tag="den", name=f"denb_{h}")
                dend_bcast = bass.AP(
                    tensor=dend.tensor,
                    offset=dend.offset,
                    ap=[[0, 64]] + list(dend.ap[1:]),
                )
                nc.sync.dma_start(denb, dend_bcast)
                denr = den_pool.tile([64, N], F32, tag="den", name=f"denr_{h}")
                return (raw, denb, denr)

            def emit_norm_mult(p, i, st, sl=slice(0, N)):
                raw, denb, denr = st
                nc.vector.reciprocal_approx_fast(out=denr[:, sl], in_=denb[:, sl])
                nc.vector.tensor_tensor(
                    aoT_sb[64 * i : 64 * i + 64, p, sl],
                    raw[0:64, sl],
                    denr[:, sl],
                    MULT,
                )

            def emit_norm(p, i, raw):
                emit_norm_mult(p, i, emit_norm_bounce(p, i, raw))

            # proj helpers (epilogue, pairwise m-tiles in pool A)
            def proj_partial(psy, m, d_range, with_bias):
                for n0, nsz in ((0, 512), (512, 256)):
                    sl = slice(n0, n0 + nsz)
                    for d in d_range:
                        nc.tensor.matmul(
                            psy[:, sl],
                            lhsT=aoT_sb[:, d, m * 128 : (m + 1) * 128],
                            rhs=wproj_sb[:, d, sl],
                            start=(d == 0),
                            stop=False,
                        )
                    if with_bias:
                        nc.tensor.matmul(
                            psy[:, sl],
                            lhsT=ones1[0:1, 0:128],
                            rhs=bp_sb[0:1, sl],
                            start=False,
                            stop=True,
                        )

            def proj_finish(psy, m):
                ysb = y_pool.tile([128, D], FP16, tag="ysb", name=f"ysb_{m}")
                nc.vector.tensor_copy(ysb, psy[:, 0:D])
                nc.sync.dma_start(y_d[m * 128 : (m + 1) * 128, :], ysb)

            # ---- prologue: pair-0 qk tiles half-by-half so the first
            # scores/exps start as soon as the qh0 x-halves land (ACT's
            # ~110us exp stream is the near-critical path).  The prologue
            # psq accumulators live in pool B so the scores' pss rotation
            # in pool A is never blocked behind a late cast. ----
            emit_qk_matmuls(0, nc.sync, qhs=(0,), pool=psB)
            emit_qk_matmuls(6, nc.sync, qhs=(0,), pool=psB)
            emit_qk_cast(0, qhs=(0,))
            emit_qk_cast(6, qhs=(0,))
            emit_scores_kt(0, 0, qhs=(0,))
            emit_scores_kt(0, 1, qhs=(0,))
            emit_qk_matmuls(0, nc.sync, qhs=(1,))
            emit_qk_matmuls(6, nc.sync, qhs=(1,))
            emit_qk_cast(0, qhs=(1,))
            emit_qk_cast(6, qhs=(1,))
            emit_scores_kt(0, 0, qhs=(1,))
            emit_scores_kt(0, 1, qhs=(1,))
            emit_qk_tile(1, nc.gpsimd, pool=psB)  # q heads 2,3
            emit_v_tile(0)
            emit_qk_tile(7, nc.gpsimd, pool=psB)  # k heads 2,3
            emit_v_tile(1)

            # round 0: remaining v tiles + scores pair 0 (skewed +2)
            for j in range(2, TOK_TILES):
                emit_v_tile(j)
                emit_scores_kt(0, j)
            # r0 tail: next round's first scores BEFORE the prefetch burst
            # (their pss slots must not rotate behind the psq tiles)
            emit_scores_kt(1, 0)
            emit_scores_kt(1, 1)
            emit_qk_matmuls(2, nc.gpsimd)
            emit_qk_matmuls(8, nc.gpsimd)
            emit_qk_cast(2)
            emit_qk_cast(8)

            # rounds 1..5: attn@v pair r-1, scores pair r (skewed +2)
            for r in range(1, 6):
                pso_live = {
                    i: psB.tile([128, N], F32, tag="psB", name=f"pso_{r - 1}_{i}")
                    for i in range(2)
                }
                for kt in range(TOK_TILES):
                    emit_attnv_kt(r - 1, kt)
                    if kt < 6:
                        emit_scores_kt(r, kt + 2)
                # tail: next round's first scores IMMEDIATELY (pss slots
                # free as the last exps retire; their qk tiles were cast a
                # round ago) so ACT never idles across the boundary; then
                # the prefetch bursts; DVE order = drain0, castA, castB,
                # drain1 so both attn@v accumulators and the kt-loop's
                # first cast-dependent scores unblock just in time.
                if r < 5:
                    emit_scores_kt(r + 1, 0)
                    emit_scores_kt(r + 1, 1)
                raw0 = emit_drain(r - 1, 0)
                if r + 2 < 6:
                    emit_qk_matmuls(r + 2, nc.gpsimd)
                    emit_qk_cast(r + 2)
                    emit_qk_matmuls(6 + r + 2, nc.gpsimd)
                    emit_qk_cast(6 + r + 2)
                raw1 = emit_drain(r - 1, 1)
                emit_norm(r - 1, 0, raw0)
                emit_norm(r - 1, 1, raw1)

            # round 6: attn@v pair 5 + first proj partials
            pso_live = {
                i: psB.tile([128, N], F32, tag="psB", name=f"pso_5_{i}")
                for i in range(2)
            }
            psy0 = psA.tile([128, N], F32, tag="psA", name="psy_0")
            psy1 = psA.tile([128, N], F32, tag="psA", name="psy_1")
            proj_partial(psy0, 0, range(5), False)
            for kt in range(TOK_TILES):
                emit_attnv_kt(5, kt)
            raw0 = emit_drain(5, 0)
            raw1 = emit_drain(5, 1)
            st0 = emit_norm_bounce(5, 0, raw0)
            st1 = emit_norm_bounce(5, 1, raw1)
            proj_partial(psy1, 1, range(5), False)
            # pool B is free after the drains: fill the norm-5 latency
            # with proj partials for m=2,3 there
            psy2 = psB.tile([128, N], F32, tag="psB", name="psy_2")
            proj_partial(psy2, 2, range(5), False)
            psy3 = psB.tile([128, N], F32, tag="psB", name="psy_3")
            proj_partial(psy3, 3, range(5), False)
            # normalize pair 5 q-half by q-half: m-tiles 0..3 only need
            # the first half of aoT[:, 5, :], so their d5 chunks unblock
            # after the h0 mults
            emit_norm_mult(5, 0, st0, slice(0, 512))
            emit_norm_mult(5, 1, st1, slice(0, 512))

            # ---- output projection (m=0..3 partials already queued) ----
            for psy, m in ((psy0, 0), (psy1, 1), (psy2, 2), (psy3, 3)):
                proj_partial(psy, m, range(5, D_SUB), True)
            emit_norm_mult(5, 0, st0, slice(512, N))
            emit_norm_mult(5, 1, st1, slice(512, N))
            for psy, m in ((psy0, 0), (psy1, 1), (psy2, 2), (psy3, 3)):
                proj_finish(psy, m)
            for m, pool, tg in ((4, psA, "psA"), (5, psA, "psA"), (6, psB, "psB"), (7, psB, "psB")):
                psy = pool.tile([128, N], F32, tag=tg, name=f"psy_{m}")
                proj_partial(psy, m, range(D_SUB), True)
                proj_finish(psy, m)

    nc.compile()
    return nc


def _in_maps(x, w_qkv, b_qkv, w_proj, b_proj):
    w_qkv = np.asarray(w_qkv, dtype=np.float32)
    b_qkv = np.asarray(b_qkv, dtype=np.float32)
    w_proj = np.asarray(w_proj, dtype=np.float32)
    b_proj = np.asarray(b_proj, dtype=np.float32)
    wqk16 = np.ascontiguousarray(w_qkv[:, :F_QK], dtype=np.float16)
    wv16 = np.ascontiguousarray(w_qkv[:, F_QK:], dtype=np.float16)
    wp16 = np.ascontiguousarray(w_proj, dtype=np.float16)
    bqk_col = np.ascontiguousarray(
        b_qkv[:F_QK].reshape(12, 128).T, dtype=np.float32
    )
    bv16 = np.ascontiguousarray(b_qkv[F_QK:], dtype=np.float16)
    bp16 = np.ascontiguousarray(b_proj, dtype=np.float16)
    maps = []
    for c in range(N_CORES):
        maps.append(
            {
                "xt": np.ascontiguousarray(
                    np.asarray(x[c], dtype=np.float32).T.astype(np.float16)
                ),
                "wqk": wqk16,
                "wv": wv16,
                "wproj": wp16,
                "bqk": bqk_col,
                "bv": bv16,
                "bp": bp16,
            }
        )
    return maps


def kernel(x, w_qkv, b_qkv, w_proj, b_proj):
    global _cached_nc
    if _cached_nc is None:
        _cached_nc = _build()
    from concourse.bass_utils import run_bass_kernel_spmd

    res = run_bass_kernel_spmd(
        _cached_nc,
        _in_maps(x, w_qkv, b_qkv, w_proj, b_proj),
        list(range(N_CORES)),
    )
    return np.stack(
        [res.results[c]["y"].astype(np.float32) for c in range(N_CORES)]
    )


if __name__ == "__main__":
    rng = np.random.default_rng(0)
    x = rng.standard_normal((B, N, D), dtype=np.float32)
    w_qkv = rng.standard_normal((D, 3 * D), dtype=np.float32) * D**-0.5
    b_qkv = rng.standard_normal(3 * D).astype(np.float32) * 0.01
    w_proj = rng.standard_normal((D, D), dtype=np.float32) * D**-0.5
    b_proj = rng.standard_normal(D).astype(np.float32) * 0.01
    y = kernel(x, w_qkv, b_qkv, w_proj, b_proj)
    print(y.shape, y.dtype)


# revision 32
# speedup vs baseline: 1.0590x; 1.0506x over previous
"""Multi-head attention (B=8, N=1024, D=768, H=12) on 8 TRN2 NeuronCores.

Sharding: pure data parallel over batch — each core handles one batch
element; weights are replicated. No collectives.

v2 — dense-PE redesign (baseline was 296us, PE idle ~30% + pstate
resets after every stall):
  * fp16 operands everywhere on the PE (host-cast x/w_qkv/w_proj):
    same 1 col/cycle as f32r but half the DMA bytes (6.3MB vs 12.6MB),
    so the prologue and round-0 v-projection are no longer DMA-starved.
    (fp8 was measured in numpy: 4.4e-2 rel err — over the 2e-2 budget.)
  * qk bias folded into the DVE PSUM->SBUF cast (tensor_scalar with a
    per-partition [128,1] bias column) — kills 24 PE bias matmuls.
  * PSUM split into two fixed 2x[128,1024] pools (16KB/part total):
    A: scores halves (double-buffered at qh granularity, so the exp
       of half k never blocks the scores matmul of half k+1), the
       round-tail qk^T prefetch burst, and the epilogue proj tiles.
    B: round-0 v tiles, then attn@v accumulators (i=0/1), then shared
       with proj in the final round.
  * attn@v PSUM released by ONE DVE copy ([65,1024] -> SBUF) right
    after the last accumulation; the softmax normalization (gpsimd
    partition_broadcast of the den row + DVE reciprocal + multiply)
    runs entirely off the PE critical path.  No DRAM bounce.
  * exp per [128,1024] qh-half (96 ACT instrs, ~983ns each) paced
    against per-kt PE work; scores/attn@v/prefetch interleaved so the
    Tensor engine never idles -> stays at the 2.4GHz pstate instead of
    dropping to 1.2GHz after each stall.
"""

import sys

sys.path.insert(0, "/opt/trn_rl_repo")

import numpy as np

B, N, D, H, HD = 8, 1024, 768, 12, 64
F_QK = 2 * D  # 1536
SCALE = HD**-0.5
TOK_TILES = N // 128  # 8
D_SUB = D // 128  # 6
N_CORES = 8

_cached_nc = None


def _build():
    import concourse.tile as tile
    from concourse import bacc, mybir

    F32 = mybir.dt.float32
    FP16 = mybir.dt.float16
    EXP = mybir.ActivationFunctionType.Exp
    MULT = mybir.AluOpType.mult
    ADD = mybir.AluOpType.add

    nc = bacc.Bacc("TRN2", target_bir_lowering=False, debug=False)

    xt_d = nc.dram_tensor("xt", [D, N], FP16, kind="ExternalInput").ap()
    wqk_d = nc.dram_tensor("wqk", [D, F_QK], FP16, kind="ExternalInput").ap()
    wv_d = nc.dram_tensor("wv", [D, D], FP16, kind="ExternalInput").ap()
    wproj_d = nc.dram_tensor("wproj", [D, D], FP16, kind="ExternalInput").ap()
    bqk_d = nc.dram_tensor("bqk", [128, 12], F32, kind="ExternalInput").ap()
    bv_d = nc.dram_tensor("bv", [D], FP16, kind="ExternalInput").ap()
    bp_d = nc.dram_tensor("bp", [D], FP16, kind="ExternalInput").ap()
    y_d = nc.dram_tensor("y", [N, D], FP16, kind="ExternalOutput").ap()

    with tile.TileContext(nc) as tc:
        with (
            tc.tile_pool(name="singles", bufs=1) as singles,
            tc.tile_pool(name="qkT", bufs=7) as qkT_pool,
            tc.tile_pool(name="wqk", bufs=16) as wqk_pool,
            tc.tile_pool(name="attnT", bufs=12) as attnT_pool,
            tc.tile_pool(name="aoraw", bufs=4) as aoraw_pool,
            tc.tile_pool(name="den", bufs=4) as den_pool,
            tc.tile_pool(name="yout", bufs=3) as y_pool,
            tc.tile_pool(name="psA", bufs=2, space="PSUM") as psA,
            tc.tile_pool(name="psB", bufs=2, space="PSUM") as psB,
            tc.tile_pool(name="dram", bufs=2, space="DRAM") as dram_pool,
        ):
            # ---- resident SBUF tensors ----
            xT_sb = singles.tile([128, D_SUB, N], FP16)  # 12KB/part
            v_sb = singles.tile([128, TOK_TILES, H * 65], FP16)  # 12.2KB/part
            aoT_sb = singles.tile([128, D_SUB, N], FP16)  # 12KB/part
            wproj_sb = singles.tile([128, D_SUB, D], FP16)  # 9KB/part
            wv_sb = singles.tile([128, D_SUB, D], FP16)  # 9KB/part
            bqk_sb = singles.tile([128, 12], F32)
            bv_sb = singles.tile([1, D], FP16)
            bp_sb = singles.tile([1, D], FP16)
            ones1 = singles.tile([1, 512], FP16)
            ones16 = singles.tile([128, 96], FP16)
            ones_f = singles.tile([128, 512], F32)

            # ---- prologue DMAs, interleaved for earliest PE start ----
            # critical chain: wt(f,d) + x half (d, qh0) feed the first
            # qk^T chains; v-weights follow; wproj trails (needed last).
            xt_r = xt_d.rearrange("(o p) n -> p o n", p=128)

            def dma_wt(eng, f, d):
                wt = wqk_pool.tile([128, 128], FP16, tag="wqk", name=f"wt_{f}_{d}")
                eng.dma_start(wt, wqk_d[d * 128 : (d + 1) * 128, f * 128 : f * 128 + 128])
                return wt

            # three issue queues in parallel: sync=x halves + first wqk
            # tiles (critical chain, interleaved so the f0-qh0 matmuls can
            # start on the first arrivals), gpsimd=biases, scalar=bulk.
            # x halves + f0 weight tiles interleaved on sync (the critical
            # chain); f6 weight tiles on gpsimd (tiny, parallel issue); the
            # bulk wv/wproj LAST on sync so their transfers don't steal
            # DMA-engine bandwidth from the x stream.
            wt_tiles = {}
            nc.gpsimd.dma_start(bqk_sb, bqk_d)
            for d in range(D_SUB):
                wt_tiles[(0, d)] = dma_wt(nc.sync, 0, d)
                nc.sync.dma_start(xT_sb[:, d, 0:512], xt_r[:, d, 0:512])
                wt_tiles[(6, d)] = dma_wt(nc.gpsimd, 6, d)
            for d in range(D_SUB):
                nc.sync.dma_start(xT_sb[:, d, 512:N], xt_r[:, d, 512:N])
            nc.sync.dma_start(wv_sb, wv_d.rearrange("(o p) f -> p o f", p=128))
            nc.gpsimd.dma_start(bv_sb, bv_d[None, :])
            nc.sync.dma_start(wproj_sb, wproj_d.rearrange("(o p) f -> p o f", p=128))
            nc.gpsimd.dma_start(bp_sb, bp_d[None, :])

            nc.vector.memset(ones_f, 1.0)
            nc.vector.tensor_copy(ones1, ones_f[0:1, :])
            nc.vector.tensor_copy(ones16, ones_f[:, 0:96])
            # ones columns of [v | 1] slots
            v_ones_view = v_sb.rearrange("p s (h c) -> p s h c", c=65)[:, :, :, 64]
            nc.vector.tensor_copy(
                v_ones_view, ones16.rearrange("p (s h) -> p s h", s=8)
            )

            qk_tiles = {}

            # ---- qk^T: one 128-feature tile (f in 0..11), fp16 out ----
            # bias folded into the PSUM->SBUF cast (per-partition add).
            # matmuls and cast are separately emittable so round tails can
            # order the DVE queue as [drains, casts] — the drains gate the
            # next round's attn@v.
            qk_psq = {}
            qk_cast_left = {}

            def emit_qk_matmuls(f, wt_eng, qhs=(0, 1), pool=None):
                if (f, 0) not in wt_tiles:
                    for d in range(D_SUB):
                        wt_tiles[(f, d)] = dma_wt(wt_eng, f, d)
                if f not in qk_psq:
                    pool = pool or psA
                    tg = "psA" if pool is psA else "psB"
                    qk_psq[f] = pool.tile([128, N], F32, tag=tg, name=f"psq_{f}")
                    qk_cast_left[f] = 2
                psq = qk_psq[f]
                for qh in qhs:
                    sl = slice(qh * 512, (qh + 1) * 512)
                    for d in range(D_SUB):
                        nc.tensor.matmul(
                            psq[:, sl],
                            lhsT=wt_tiles[(f, d)],
                            rhs=xT_sb[:, d, sl],
                            start=(d == 0),
                            stop=(d == D_SUB - 1),
                        )

            def emit_qk_cast(f, qhs=(0, 1)):
                # per-qh-half casts: a half only needs the x halves that fed
                # it, so the first scores/exps can start before all of x has
                # landed
                psq = qk_psq[f]
                if f not in qk_tiles:
                    qk_tiles[f] = qkT_pool.tile(
                        [128, N], FP16, tag="qkT", name=f"qkT_{f}"
                    )
                qt = qk_tiles[f]
                with tc.high_priority():
                    for qh in qhs:
                        sl = slice(qh * 512, (qh + 1) * 512)
                        nc.vector.tensor_scalar(
                            out=qt[:, sl], in0=psq[:, sl],
                            scalar1=bqk_sb[:, f : f + 1], scalar2=None, op0=ADD,
                        )
                        qk_cast_left[f] -= 1
                if qk_cast_left[f] == 0:
                    del qk_psq[f]

            def emit_qk_tile(f, wt_eng, pool=None):
                emit_qk_matmuls(f, wt_eng, pool=pool)
                emit_qk_cast(f)

            # ---- v m-tile: natural layout, scattered into 65-slots ----
            def emit_v_tile(m):
                psv = psB.tile([128, N], F32, tag="psB", name=f"psv_{m}")
                for n0, nsz in ((0, 512), (512, 256)):
                    sl = slice(n0, n0 + nsz)
                    for d in range(D_SUB):
                        nc.tensor.matmul(
                            psv[:, sl],
                            lhsT=xT_sb[:, d, m * 128 : (m + 1) * 128],
                            rhs=wv_sb[:, d, sl],
                            start=(d == 0),
                            stop=False,
                        )
                    nc.tensor.matmul(
                        psv[:, sl],
                        lhsT=ones1[0:1, 0:128],
                        rhs=bv_sb[0:1, sl],
                        start=False,
                        stop=True,
                    )
                nc.vector.tensor_copy(
                    v_sb[:, m, :].rearrange("p (h c) -> p h c", c=65)[:, :, 0:64],
                    psv[:, 0:D].rearrange("p (h c) -> p h c", c=64),
                )

            # ---- attention rounds, software-pipelined over head pairs ----
            attn_tiles = {}  # (pair, kt) -> [128, 2048] fp16: [A0|B0|A1|B1]
            pso_live = {}

            def emit_scores_half(p, kt, qh):
                # the exp stream on ACT is the near-critical path: keep the
                # whole scores->exp chain at the front of the scheduler's
                # priority heap so it is never deferred behind bulk matmuls
                qT = qk_tiles[p]
                kT = qk_tiles[6 + p]
                pss = psA.tile([128, N], F32, tag="psA", name=f"pss_{p}_{kt}_{qh}")
                with tc.high_priority():
                    for i in range(2):
                        pb = slice(64 * i, 64 * i + 64)
                        nc.tensor.matmul(
                            pss[:, i * 512 : i * 512 + 512],
                            lhsT=kT[pb, kt * 128 : (kt + 1) * 128],
                            rhs=qT[pb, qh * 512 : (qh + 1) * 512],
                            start=True,
                            stop=True,
                        )
                    at = attn_tiles[(p, kt)]
                    nc.scalar.activation(
                        at[:, qh * 1024 : (qh + 1) * 1024], pss, func=EXP, scale=SCALE
                    )

            def emit_scores_kt(p, kt, qhs=(0, 1)):
                if (p, kt) not in attn_tiles:
                    attn_tiles[(p, kt)] = attnT_pool.tile(
                        [128, 2 * N], FP16, tag="attnT", name=f"at_{p}_{kt}"
                    )
                for qh in qhs:
                    emit_scores_half(p, kt, qh)

            def emit_attnv_kt(p, kt):
                at = attn_tiles[(p, kt)]
                for i in range(2):
                    h = 2 * p + i
                    for qh in range(2):
                        osl = slice(qh * 512, (qh + 1) * 512)
                        isl = slice(qh * 1024 + i * 512, qh * 1024 + i * 512 + 512)
                        nc.tensor.matmul(
                            pso_live[i][0:65, osl],
                            lhsT=v_sb[:, kt, h * 65 : h * 65 + 65],
                            rhs=at[:, isl],
                            start=(kt == 0),
                            stop=(kt == TOK_TILES - 1),
                        )
                if kt == TOK_TILES - 1:
                    attn_tiles.pop((p, kt))
                else:
                    del attn_tiles[(p, kt)]

            def emit_drain(p, i):
                # single DVE copy releases the PSUM accumulator fast; high
                # priority so it isn't queued behind other DVE work (it
                # gates the next round's attn@v accumulators)
                h = 2 * p + i
                raw = aoraw_pool.tile([65, N], F32, tag="aoraw", name=f"raw_{h}")
                with tc.high_priority():
                    nc.vector.tensor_copy(raw, pso_live[i][0:65, :])
                return raw

            def emit_norm_bounce(p, i, raw):
                # den row -> DRAM-bounce broadcast to 64 partitions
                # (partition-step-0 read is legal from DRAM; the gpsimd
                # partition_broadcast reads physical partition 0 on HW)
                import concourse.bass as bass

                h = 2 * p + i
                dend = dram_pool.tile([1, N], F32, tag="dend", name=f"dend_{h}")
                nc.sync.dma_start(dend, raw[64:65, :])
                denb = den_pool.tile([64, N], F32, tag="den", name=f"denb_{h}")
                dend_bcast = bass.AP(
                    tensor=dend.tensor,
                    offset=dend.offset,
                    ap=[[0, 64]] + list(dend.ap[1:]),
                )
                nc.sync.dma_start(denb, dend_bcast)
                denr = den_pool.tile([64, N], F32, tag="den", name=f"denr_{h}")
                return (raw, denb, denr)

            def emit_norm_mult(p, i, st, sl=slice(0, N)):
                raw, denb, denr = st
                nc.vector.reciprocal_approx_fast(out=denr[:, sl], in_=denb[:, sl])
                nc.vector.tensor_tensor(
                    aoT_sb[64 * i : 64 * i + 64, p, sl],
                    raw[0:64, sl],
                    denr[:, sl],
                    MULT,
                )

            def emit_norm(p, i, raw):
                emit_norm_mult(p, i, emit_norm_bounce(p, i, raw))

            # proj helpers (epilogue, pairwise m-tiles in pool A)
            def proj_partial(psy, m, d_range, with_bias):
                for n0, nsz in ((0, 512), (512, 256)):
                    sl = slice(n0, n0 + nsz)
                    for d in d_range:
                        nc.tensor.matmul(
                            psy[:, sl],
                            lhsT=aoT_sb[:, d, m * 128 : (m + 1) * 128],
                            rhs=wproj_sb[:, d, sl],
                            start=(d == 0),
                            stop=False,
                        )
                    if with_bias:
                        nc.tensor.matmul(
                            psy[:, sl],
                            lhsT=ones1[0:1, 0:128],
                            rhs=bp_sb[0:1, sl],
                            start=False,
                            stop=True,
                        )

            def proj_finish(psy, m):
                ysb = y_pool.tile([128, D], FP16, tag="ysb", name=f"ysb_{m}")
                nc.vector.tensor_copy(ysb, psy[:, 0:D])
                nc.sync.dma_start(y_d[m * 128 : (m + 1) * 128, :], ysb)

            # ---- prologue: pair-0 qk tiles half-by-half so the first
            # scores/exps start as soon as the qh0 x-halves land (ACT's
            # ~110us exp stream is the near-critical path).  The prologue
            # psq accumulators live in pool B so the scores' pss rotation
            # in pool A is never blocked behind a late cast. ----
            emit_qk_matmuls(0, nc.sync, qhs=(0,), pool=psB)
            emit_qk_matmuls(6, nc.sync, qhs=(0,), pool=psB)
            emit_qk_cast(0, qhs=(0,))
            emit_qk_cast(6, qhs=(0,))
            emit_scores_kt(0, 0, qhs=(0,))
            emit_scores_kt(0, 1, qhs=(0,))
            emit_qk_matmuls(0, nc.sync, qhs=(1,))
            emit_qk_matmuls(6, nc.sync, qhs=(1,))
            emit_qk_cast(0, qhs=(1,))
            emit_qk_cast(6, qhs=(1,))
            emit_scores_kt(0, 0, qhs=(1,))
            emit_scores_kt(0, 1, qhs=(1,))
            emit_qk_tile(1, nc.gpsimd, pool=psB)  # q heads 2,3
            emit_v_tile(0)
            emit_qk_tile(7, nc.gpsimd, pool=psB)  # k heads 2,3
            emit_v_tile(1)

            # round 0: remaining v tiles + scores pair 0 (skewed +2)
            for j in range(2, TOK_TILES):
                emit_v_tile(j)
                emit_scores_kt(0, j)
            # r0 tail: next round's first scores BEFORE the prefetch burst
            # (their pss slots must not rotate behind the psq tiles)
            emit_scores_kt(1, 0)
            emit_scores_kt(1, 1)
            emit_qk_matmuls(2, nc.gpsimd)
            emit_qk_matmuls(8, nc.gpsimd)
            emit_qk_cast(2)
            emit_qk_cast(8)

            # rounds 1..5: attn@v pair r-1, scores pair r (skewed +2)
            for r in range(1, 6):
                pso_live = {
                    i: psB.tile([128, N], F32, tag="psB", name=f"pso_{r - 1}_{i}")
                    for i in range(2)
                }
                for kt in range(TOK_TILES):
                    emit_attnv_kt(r - 1, kt)
                    if kt < 6:
                        emit_scores_kt(r, kt + 2)
                # tail: next round's first scores IMMEDIATELY (pss slots
                # free as the last exps retire; their qk tiles were cast a
                # round ago) so ACT never idles across the boundary; then
                # the prefetch bursts; DVE order = drain0, castA, castB,
                # drain1 so both attn@v accumulators and the kt-loop's
                # first cast-dependent scores unblock just in time.
                if r < 5:
                    emit_scores_kt(r + 1, 0)
                    emit_scores_kt(r + 1, 1)
                raw0 = emit_drain(r - 1, 0)
                if r + 2 < 6:
                    emit_qk_matmuls(r + 2, nc.gpsimd)
                    emit_qk_cast(r + 2)
                    emit_qk_matmuls(6 + r + 2, nc.gpsimd)
                    emit_qk_cast(6 + r + 2)
                raw1 = emit_drain(r - 1, 1)
                emit_norm(r - 1, 0, raw0)
                emit_norm(r - 1, 1, raw1)

            # round 6: attn@v pair 5 + first proj partials
            pso_live = {
                i: psB.tile([128, N], F32, tag="psB", name=f"pso_5_{i}")
                for i in range(2)
            }
            psy0 = psA.tile([128, N], F32, tag="psA", name="psy_0")
            psy1 = psA.tile([128, N], F32, tag="psA", name="psy_1")
            proj_partial(psy0, 0, range(5), False)
            for kt in range(TOK_TILES):
                emit_attnv_kt(5, kt)
            # pair-5 drain + den bounce split at q-halves: the h0 chain
            # (copy -> DRAM bounce -> recip -> mult) starts right after the
            # last attn@v instead of behind full-row copies, so the
            # projection's d5 chunks for m=0..3 unblock ~5us earlier
            import concourse.bass as bass

            raw0 = aoraw_pool.tile([65, N], F32, tag="aoraw", name="raw_10")
            raw1 = aoraw_pool.tile([65, N], F32, tag="aoraw", name="raw_11")
            denb5 = {i: den_pool.tile([64, N], F32, tag="den", name=f"denb5_{i}") for i in range(2)}
            denr5 = {i: den_pool.tile([64, N], F32, tag="den", name=f"denr5_{i}") for i in range(2)}

            def bounce5(i, raw, qh):
                sl = slice(qh * 512, (qh + 1) * 512)
                dend = dram_pool.tile(
                    [1, 512], F32, tag="dendh", name=f"dendh_{i}_{qh}"
                )
                nc.sync.dma_start(dend, raw[64:65, sl])
                dend_bcast = bass.AP(
                    tensor=dend.tensor,
                    offset=dend.offset,
                    ap=[[0, 64]] + list(dend.ap[1:]),
                )
                nc.sync.dma_start(denb5[i][:, sl], dend_bcast)

            def norm5_mult(i, sl):
                nc.vector.reciprocal_approx_fast(
                    out=denr5[i][:, sl], in_=denb5[i][:, sl]
                )
                nc.vector.tensor_tensor(
                    aoT_sb[64 * i : 64 * i + 64, 5, sl],
                    (raw0 if i == 0 else raw1)[0:64, sl],
                    denr5[i][:, sl],
                    MULT,
                )

            with tc.high_priority():
                nc.vector.tensor_copy(raw0[:, 0:512], pso_live[0][0:65, 0:512])
            bounce5(0, raw0, 0)
            with tc.high_priority():
                nc.vector.tensor_copy(raw1[:, 0:512], pso_live[1][0:65, 0:512])
            bounce5(1, raw1, 0)
            with tc.high_priority():
                nc.vector.tensor_copy(raw0[:, 512:N], pso_live[0][0:65, 512:N])
                nc.vector.tensor_copy(raw1[:, 512:N], pso_live[1][0:65, 512:N])
            bounce5(0, raw0, 1)
            bounce5(1, raw1, 1)
            proj_partial(psy1, 1, range(5), False)
            # pool B is free after the raw copies: fill the norm-5 latency
            # with proj partials for m=2,3 there
            psy2 = psB.tile([128, N], F32, tag="psB", name="psy_2")
            proj_partial(psy2, 2, range(5), False)
            psy3 = psB.tile([128, N], F32, tag="psB", name="psy_3")
            proj_partial(psy3, 3, range(5), False)
            norm5_mult(0, slice(0, 512))
            norm5_mult(1, slice(0, 512))

            # ---- output projection (m=0..3 partials already queued) ----
            for psy, m in ((psy0, 0), (psy1, 1), (psy2, 2), (psy3, 3)):
                proj_partial(psy, m, range(5, D_SUB), True)
            norm5_mult(0, slice(512, N))
            norm5_mult(1, slice(512, N))
            for psy, m in ((psy0, 0), (psy1, 1), (psy2, 2), (psy3, 3)):
                proj_finish(psy, m)
            for m, pool, tg in ((4, psA, "psA"), (5, psA, "psA"), (6, psB, "psB"), (7, psB, "psB")):
                psy = pool.tile([128, N], F32, tag=tg, name=f"psy_{m}")
                proj_partial(psy, m, range(D_SUB), True)
                proj_finish(psy, m)

    nc.compile()
    return nc


def _in_maps(x, w_qkv, b_qkv, w_proj, b_proj):
    w_qkv = np.asarray(w_qkv, dtype=np.float32)
    b_qkv = np.asarray(b_qkv, dtype=np.float32)
    w_proj = np.asarray(w_proj, dtype=np.float32)
    b_proj = np.asarray(b_proj, dtype=np.float32)
    wqk16 = np.ascontiguousarray(w_qkv[:, :F_QK], dtype=np.float16)
    wv16 = np.ascontiguousarray(w_qkv[:, F_QK:], dtype=np.float16)
    wp16 = np.ascontiguousarray(w_proj, dtype=np.float16)
    bqk_col = np.ascontiguousarray(
        b_qkv[:F_QK].reshape(12, 128).T, dtype=np.float32
    )
    bv16 = np.ascontiguousarray(b_qkv[F_QK:], dtype=np.float16)
    bp16 = np.ascontiguousarray(b_proj, dtype=np.float16)
    maps = []
    for c in range(N_CORES):
        maps.append(
            {
                "xt": np.ascontiguousarray(
                    np.asarray(x[c], dtype=np.float32).T.astype(np.float16)
                ),
                "wqk": wqk16,
                "wv": wv16,
                "wproj": wp16,
                "bqk": bqk_col,
                "bv": bv16,
                "bp": bp16,
            }
        )
    return maps


def kernel(x, w_qkv, b_qkv, w_proj, b_proj):
    global _cached_nc
    if _cached_nc is None:
        _cached_nc = _build()
    from concourse.bass_utils import run_bass_kernel_spmd

    res = run_bass_kernel_spmd(
        _cached_nc,
        _in_maps(x, w_qkv, b_qkv, w_proj, b_proj),
        list(range(N_CORES)),
    )
    return np.stack(
        [res.results[c]["y"].astype(np.float32) for c in range(N_CORES)]
    )


if __name__ == "__main__":
    rng = np.random.default_rng(0)
    x = rng.standard_normal((B, N, D), dtype=np.float32)
    w_qkv = rng.standard_normal((D, 3 * D), dtype=np.float32) * D**-0.5
    b_qkv = rng.standard_normal(3 * D).astype(np.float32) * 0.01
    w_proj = rng.standard_normal((D, D), dtype=np.float32) * D**-0.5
    b_proj = rng.standard_normal(D).astype(np.float32) * 0.01
    y = kernel(x, w_qkv, b_qkv, w_proj, b_proj)
    print(y.shape, y.dtype)


# revision 36
# speedup vs baseline: 1.0982x; 1.0370x over previous
"""Multi-head attention (B=8, N=1024, D=768, H=12) on 8 TRN2 NeuronCores.

Sharding: pure data parallel over batch — each core handles one batch
element; weights are replicated. No collectives.

v2 — dense-PE redesign (baseline was 296us, PE idle ~30% + pstate
resets after every stall):
  * fp16 operands everywhere on the PE (host-cast x/w_qkv/w_proj):
    same 1 col/cycle as f32r but half the DMA bytes (6.3MB vs 12.6MB),
    so the prologue and round-0 v-projection are no longer DMA-starved.
    (fp8 was measured in numpy: 4.4e-2 rel err — over the 2e-2 budget.)
  * qk bias folded into the DVE PSUM->SBUF cast (tensor_scalar with a
    per-partition [128,1] bias column) — kills 24 PE bias matmuls.
  * PSUM split into two fixed 2x[128,1024] pools (16KB/part total):
    A: scores halves (double-buffered at qh granularity, so the exp
       of half k never blocks the scores matmul of half k+1), the
       round-tail qk^T prefetch burst, and the epilogue proj tiles.
    B: round-0 v tiles, then attn@v accumulators (i=0/1), then shared
       with proj in the final round.
  * attn@v PSUM released by ONE DVE copy ([65,1024] -> SBUF) right
    after the last accumulation; the softmax normalization (gpsimd
    partition_broadcast of the den row + DVE reciprocal + multiply)
    runs entirely off the PE critical path.  No DRAM bounce.
  * exp per [128,1024] qh-half (96 ACT instrs, ~983ns each) paced
    against per-kt PE work; scores/attn@v/prefetch interleaved so the
    Tensor engine never idles -> stays at the 2.4GHz pstate instead of
    dropping to 1.2GHz after each stall.
"""

import sys

sys.path.insert(0, "/opt/trn_rl_repo")

import numpy as np

B, N, D, H, HD = 8, 1024, 768, 12, 64
F_QK = 2 * D  # 1536
SCALE = HD**-0.5
TOK_TILES = N // 128  # 8
D_SUB = D // 128  # 6
N_CORES = 8

_cached_nc = None


def _build():
    import concourse.tile as tile
    from concourse import bacc, mybir

    F32 = mybir.dt.float32
    FP16 = mybir.dt.float16
    EXP = mybir.ActivationFunctionType.Exp
    MULT = mybir.AluOpType.mult
    ADD = mybir.AluOpType.add

    nc = bacc.Bacc("TRN2", target_bir_lowering=False, debug=False)

    xt_d = nc.dram_tensor("xt", [D, N], FP16, kind="ExternalInput").ap()
    wqk_d = nc.dram_tensor("wqk", [D, F_QK], FP16, kind="ExternalInput").ap()
    wv_d = nc.dram_tensor("wv", [D, D], FP16, kind="ExternalInput").ap()
    wproj_d = nc.dram_tensor("wproj", [D, D], FP16, kind="ExternalInput").ap()
    bqk_d = nc.dram_tensor("bqk", [128, 12], F32, kind="ExternalInput").ap()
    bv_d = nc.dram_tensor("bv", [D], FP16, kind="ExternalInput").ap()
    bp_d = nc.dram_tensor("bp", [D], FP16, kind="ExternalInput").ap()
    y_d = nc.dram_tensor("y", [N, D], FP16, kind="ExternalOutput").ap()

    with tile.TileContext(nc) as tc:
        with (
            tc.tile_pool(name="singles", bufs=1) as singles,
            tc.tile_pool(name="qkT", bufs=7) as qkT_pool,
            tc.tile_pool(name="wqk", bufs=16) as wqk_pool,
            tc.tile_pool(name="attnT", bufs=12) as attnT_pool,
            tc.tile_pool(name="aoraw", bufs=4) as aoraw_pool,
            tc.tile_pool(name="den", bufs=4) as den_pool,
            tc.tile_pool(name="yout", bufs=3) as y_pool,
            tc.tile_pool(name="psA", bufs=2, space="PSUM") as psA,
            tc.tile_pool(name="psB", bufs=2, space="PSUM") as psB,
            tc.tile_pool(name="dram", bufs=2, space="DRAM") as dram_pool,
        ):
            # ---- resident SBUF tensors ----
            xT_sb = singles.tile([128, D_SUB, N], FP16)  # 12KB/part
            v_sb = singles.tile([128, TOK_TILES, H * 65], FP16)  # 12.2KB/part
            aoT_sb = singles.tile([128, D_SUB, N], FP16)  # 12KB/part
            wproj_sb = singles.tile([128, D_SUB, D], FP16)  # 9KB/part
            wv_sb = singles.tile([128, D_SUB, D], FP16)  # 9KB/part
            bqk_sb = singles.tile([128, 12], F32)
            bv_sb = singles.tile([1, D], FP16)
            bp_sb = singles.tile([1, D], FP16)
            ones1 = singles.tile([1, 512], FP16)
            ones16 = singles.tile([128, 96], FP16)
            ones_f = singles.tile([128, 512], F32)

            # ---- prologue DMAs, interleaved for earliest PE start ----
            # critical chain: wt(f,d) + x half (d, qh0) feed the first
            # qk^T chains; v-weights follow; wproj trails (needed last).
            xt_r = xt_d.rearrange("(o p) n -> p o n", p=128)

            def dma_wt(eng, f, d):
                wt = wqk_pool.tile([128, 128], FP16, tag="wqk", name=f"wt_{f}_{d}")
                eng.dma_start(wt, wqk_d[d * 128 : (d + 1) * 128, f * 128 : f * 128 + 128])
                return wt

            # three issue queues in parallel: sync=x halves + first wqk
            # tiles (critical chain, interleaved so the f0-qh0 matmuls can
            # start on the first arrivals), gpsimd=biases, scalar=bulk.
            # x halves + f0 weight tiles interleaved on sync (the critical
            # chain); f6 weight tiles on gpsimd (tiny, parallel issue); the
            # bulk wv/wproj LAST on sync so their transfers don't steal
            # DMA-engine bandwidth from the x stream.
            wt_tiles = {}
            nc.gpsimd.dma_start(bqk_sb, bqk_d)
            for d in range(D_SUB):
                wt_tiles[(0, d)] = dma_wt(nc.sync, 0, d)
                nc.sync.dma_start(xT_sb[:, d, 0:512], xt_r[:, d, 0:512])
                wt_tiles[(6, d)] = dma_wt(nc.gpsimd, 6, d)
            for d in range(D_SUB):
                nc.sync.dma_start(xT_sb[:, d, 512:N], xt_r[:, d, 512:N])
            nc.sync.dma_start(wv_sb, wv_d.rearrange("(o p) f -> p o f", p=128))
            nc.gpsimd.dma_start(bv_sb, bv_d[None, :])
            nc.sync.dma_start(wproj_sb, wproj_d.rearrange("(o p) f -> p o f", p=128))
            nc.gpsimd.dma_start(bp_sb, bp_d[None, :])

            nc.vector.memset(ones_f, 1.0)
            nc.vector.tensor_copy(ones1, ones_f[0:1, :])
            nc.vector.tensor_copy(ones16, ones_f[:, 0:96])
            # ones columns of [v | 1] slots
            v_ones_view = v_sb.rearrange("p s (h c) -> p s h c", c=65)[:, :, :, 64]
            nc.vector.tensor_copy(
                v_ones_view, ones16.rearrange("p (s h) -> p s h", s=8)
            )

            qk_tiles = {}

            # ---- qk^T: one 128-feature tile (f in 0..11), fp16 out ----
            # bias folded into the PSUM->SBUF cast (per-partition add).
            # matmuls and cast are separately emittable so round tails can
            # order the DVE queue as [drains, casts] — the drains gate the
            # next round's attn@v.
            qk_psq = {}
            qk_cast_left = {}

            def emit_qk_matmuls(f, wt_eng, qhs=(0, 1), pool=None):
                if (f, 0) not in wt_tiles:
                    for d in range(D_SUB):
                        wt_tiles[(f, d)] = dma_wt(wt_eng, f, d)
                if f not in qk_psq:
                    pool = pool or psA
                    tg = "psA" if pool is psA else "psB"
                    qk_psq[f] = pool.tile([128, N], F32, tag=tg, name=f"psq_{f}")
                    qk_cast_left[f] = 2
                psq = qk_psq[f]
                for qh in qhs:
                    sl = slice(qh * 512, (qh + 1) * 512)
                    for d in range(D_SUB):
                        nc.tensor.matmul(
                            psq[:, sl],
                            lhsT=wt_tiles[(f, d)],
                            rhs=xT_sb[:, d, sl],
                            start=(d == 0),
                            stop=(d == D_SUB - 1),
                        )

            def emit_qk_cast(f, qhs=(0, 1)):
                # per-qh-half casts: a half only needs the x halves that fed
                # it, so the first scores/exps can start before all of x has
                # landed
                psq = qk_psq[f]
                if f not in qk_tiles:
                    qk_tiles[f] = qkT_pool.tile(
                        [128, N], FP16, tag="qkT", name=f"qkT_{f}"
                    )
                qt = qk_tiles[f]
                with tc.high_priority():
                    for qh in qhs:
                        sl = slice(qh * 512, (qh + 1) * 512)
                        nc.vector.tensor_scalar(
                            out=qt[:, sl], in0=psq[:, sl],
                            scalar1=bqk_sb[:, f : f + 1], scalar2=None, op0=ADD,
                        )
                        qk_cast_left[f] -= 1
                if qk_cast_left[f] == 0:
                    del qk_psq[f]

            def emit_qk_tile(f, wt_eng, pool=None):
                emit_qk_matmuls(f, wt_eng, pool=pool)
                emit_qk_cast(f)

            # ---- v m-tile: natural layout, scattered into 65-slots ----
            def emit_v_tile(m):
                psv = psB.tile([128, N], F32, tag="psB", name=f"psv_{m}")
                for n0, nsz in ((0, 512), (512, 256)):
                    sl = slice(n0, n0 + nsz)
                    for d in range(D_SUB):
                        nc.tensor.matmul(
                            psv[:, sl],
                            lhsT=xT_sb[:, d, m * 128 : (m + 1) * 128],
                            rhs=wv_sb[:, d, sl],
                            start=(d == 0),
                            stop=False,
                        )
                    nc.tensor.matmul(
                        psv[:, sl],
                        lhsT=ones1[0:1, 0:128],
                        rhs=bv_sb[0:1, sl],
                        start=False,
                        stop=True,
                    )
                nc.vector.tensor_copy(
                    v_sb[:, m, :].rearrange("p (h c) -> p h c", c=65)[:, :, 0:64],
                    psv[:, 0:D].rearrange("p (h c) -> p h c", c=64),
                )

            # ---- attention rounds, software-pipelined over head pairs ----
            attn_tiles = {}  # (pair, kt) -> [128, 2048] fp16: [A0|B0|A1|B1]
            pso_live = {}

            def emit_scores_half(p, kt, qh):
                # the exp stream on ACT is the near-critical path: keep the
                # whole scores->exp chain at the front of the scheduler's
                # priority heap so it is never deferred behind bulk matmuls
                qT = qk_tiles[p]
                kT = qk_tiles[6 + p]
                pss = psA.tile([128, N], F32, tag="psA", name=f"pss_{p}_{kt}_{qh}")
                with tc.high_priority():
                    for i in range(2):
                        pb = slice(64 * i, 64 * i + 64)
                        nc.tensor.matmul(
                            pss[:, i * 512 : i * 512 + 512],
                            lhsT=kT[pb, kt * 128 : (kt + 1) * 128],
                            rhs=qT[pb, qh * 512 : (qh + 1) * 512],
                            start=True,
                            stop=True,
                        )
                    at = attn_tiles[(p, kt)]
                    nc.scalar.activation(
                        at[:, qh * 1024 : (qh + 1) * 1024], pss, func=EXP, scale=SCALE
                    )

            def emit_scores_kt(p, kt, qhs=(0, 1)):
                if (p, kt) not in attn_tiles:
                    attn_tiles[(p, kt)] = attnT_pool.tile(
                        [128, 2 * N], FP16, tag="attnT", name=f"at_{p}_{kt}"
                    )
                for qh in qhs:
                    emit_scores_half(p, kt, qh)

            def emit_attnv_kt(p, i, kt, pso):
                at = attn_tiles[(p, kt)]
                h = 2 * p + i
                for qh in range(2):
                    osl = slice(qh * 512, (qh + 1) * 512)
                    isl = slice(qh * 1024 + i * 512, qh * 1024 + i * 512 + 512)
                    nc.tensor.matmul(
                        pso[0:65, osl],
                        lhsT=v_sb[:, kt, h * 65 : h * 65 + 65],
                        rhs=at[:, isl],
                        start=(kt == 0),
                        stop=(kt == TOK_TILES - 1),
                    )
                if i == 1:
                    del attn_tiles[(p, kt)]

            def emit_drain(p, i, pso):
                # single DVE copy releases the PSUM accumulator fast; high
                # priority so it isn't queued behind other DVE work (it
                # gates the next pass's attn@v accumulator)
                h = 2 * p + i
                raw = aoraw_pool.tile([65, N], F32, tag="aoraw", name=f"raw_{h}")
                with tc.high_priority():
                    nc.vector.tensor_copy(raw, pso[0:65, :])
                return raw

            def emit_norm_bounce(p, i, raw):
                # den row -> DRAM-bounce broadcast to 64 partitions
                # (partition-step-0 read is legal from DRAM; the gpsimd
                # partition_broadcast reads physical partition 0 on HW)
                import concourse.bass as bass

                h = 2 * p + i
                dend = dram_pool.tile([1, N], F32, tag="dend", name=f"dend_{h}")
                nc.sync.dma_start(dend, raw[64:65, :])
                denb = den_pool.tile([64, N], F32, tag="den", name=f"denb_{h}")
                dend_bcast = bass.AP(
                    tensor=dend.tensor,
                    offset=dend.offset,
                    ap=[[0, 64]] + list(dend.ap[1:]),
                )
                nc.sync.dma_start(denb, dend_bcast)
                denr = den_pool.tile([64, N], F32, tag="den", name=f"denr_{h}")
                return (raw, denb, denr)

            def emit_norm_mult(p, i, st, sl=slice(0, N)):
                raw, denb, denr = st
                nc.vector.reciprocal_approx_fast(out=denr[:, sl], in_=denb[:, sl])
                nc.vector.tensor_tensor(
                    aoT_sb[64 * i : 64 * i + 64, p, sl],
                    raw[0:64, sl],
                    denr[:, sl],
                    MULT,
                )

            def emit_norm(p, i, raw):
                emit_norm_mult(p, i, emit_norm_bounce(p, i, raw))

            # proj helpers (epilogue, pairwise m-tiles in pool A)
            def proj_partial(psy, m, d_range, with_bias):
                for n0, nsz in ((0, 512), (512, 256)):
                    sl = slice(n0, n0 + nsz)
                    for d in d_range:
                        nc.tensor.matmul(
                            psy[:, sl],
                            lhsT=aoT_sb[:, d, m * 128 : (m + 1) * 128],
                            rhs=wproj_sb[:, d, sl],
                            start=(d == 0),
                            stop=False,
                        )
                    if with_bias:
                        nc.tensor.matmul(
                            psy[:, sl],
                            lhsT=ones1[0:1, 0:128],
                            rhs=bp_sb[0:1, sl],
                            start=False,
                            stop=True,
                        )

            def proj_finish(psy, m):
                ysb = y_pool.tile([128, D], FP16, tag="ysb", name=f"ysb_{m}")
                nc.vector.tensor_copy(ysb, psy[:, 0:D])
                nc.sync.dma_start(y_d[m * 128 : (m + 1) * 128, :], ysb)

            # ---- prologue: pair-0 qk tiles half-by-half so the first
            # scores/exps start as soon as the qh0 x-halves land (ACT's
            # ~110us exp stream is the near-critical path).  The prologue
            # psq accumulators live in pool B so the scores' pss rotation
            # in pool A is never blocked behind a late cast. ----
            emit_qk_matmuls(0, nc.sync, qhs=(0,), pool=psB)
            emit_qk_matmuls(6, nc.sync, qhs=(0,), pool=psB)
            emit_qk_cast(0, qhs=(0,))
            emit_qk_cast(6, qhs=(0,))
            emit_scores_kt(0, 0, qhs=(0,))
            emit_scores_kt(0, 1, qhs=(0,))
            emit_qk_matmuls(0, nc.sync, qhs=(1,))
            emit_qk_matmuls(6, nc.sync, qhs=(1,))
            emit_qk_cast(0, qhs=(1,))
            emit_qk_cast(6, qhs=(1,))
            emit_scores_kt(0, 0, qhs=(1,))
            emit_scores_kt(0, 1, qhs=(1,))
            emit_qk_tile(1, nc.gpsimd, pool=psB)  # q heads 2,3
            emit_v_tile(0)
            emit_qk_tile(7, nc.gpsimd, pool=psB)  # k heads 2,3
            emit_v_tile(1)

            # round 0: remaining v tiles + scores pair 0 (skewed +2), with
            # the pair-2 qk prefetch interleaved in pool B alongside the v
            # accumulators (one psv + one psq live at a time)
            for j in range(2, TOK_TILES):
                emit_v_tile(j)
                emit_scores_kt(0, j)
                if j == 3:
                    emit_qk_matmuls(2, nc.gpsimd, qhs=(0,), pool=psB)
                elif j == 4:
                    emit_qk_matmuls(2, nc.gpsimd, qhs=(1,))
                    emit_qk_cast(2)
                elif j == 5:
                    emit_qk_matmuls(8, nc.gpsimd, qhs=(0,), pool=psB)
                elif j == 6:
                    emit_qk_matmuls(8, nc.gpsimd, qhs=(1,))
                    emit_qk_cast(8)
            emit_scores_kt(1, 0)
            emit_scores_kt(1, 1)

            # rounds 1..5: attn@v pair r-1 one head (i) at a time — a single
            # PSUM accumulator per pass frees the second pool-B slot for the
            # qk^T prefetch to run INSIDE the loop as PE filler; scores for
            # pair r (and the next round's first two) spread over both
            # passes so the exp stream on ACT never starves.  No round tail.
            for r in range(1, 6):
                qk_a, qk_b = r + 2, 6 + r + 2
                has_qk = qk_a < 6
                pso0 = psB.tile([128, N], F32, tag="psB", name=f"pso_{r - 1}_0")
                for kt in range(TOK_TILES):
                    emit_attnv_kt(r - 1, 0, kt, pso0)
                    if kt % 2 == 0:
                        emit_scores_kt(r, 2 + kt // 2)
                    if has_qk:
                        if kt == 1:
                            emit_qk_matmuls(qk_a, nc.gpsimd, qhs=(0,), pool=psB)
                        elif kt == 3:
                            emit_qk_matmuls(qk_a, nc.gpsimd, qhs=(1,))
                        elif kt == 5:
                            emit_qk_cast(qk_a)
                raw0 = emit_drain(r - 1, 0, pso0)
                pso1 = psB.tile([128, N], F32, tag="psB", name=f"pso_{r - 1}_1")
                emit_norm_bounce_st = emit_norm_bounce(r - 1, 0, raw0)
                for kt in range(TOK_TILES):
                    emit_attnv_kt(r - 1, 1, kt, pso1)
                    if kt == 0:
                        emit_scores_kt(r, 6)
                    elif kt == 2:
                        emit_scores_kt(r, 7)
                    elif kt == 4 and r < 5:
                        emit_scores_kt(r + 1, 0)
                    elif kt == 6 and r < 5:
                        emit_scores_kt(r + 1, 1)
                    if has_qk:
                        if kt == 1:
                            emit_qk_matmuls(qk_b, nc.gpsimd, qhs=(0,), pool=psB)
                        elif kt == 3:
                            emit_qk_matmuls(qk_b, nc.gpsimd, qhs=(1,))
                        elif kt == 5:
                            emit_qk_cast(qk_b)
                emit_norm_mult(r - 1, 0, emit_norm_bounce_st)
                raw1 = emit_drain(r - 1, 1, pso1)
                emit_norm(r - 1, 1, raw1)

            # round 6: attn@v pair 5, two passes, proj pipelined in.  The
            # i=0 head's norm completes during the i=1 pass; i=1's drain +
            # den bounce are split at q-halves so the projection's d5
            # chunks for m=0..3 unblock right after the h0 mult.
            import concourse.bass as bass

            pso0 = psB.tile([128, N], F32, tag="psB", name="pso_5_0")
            psy0 = psA.tile([128, N], F32, tag="psA", name="psy_0")
            psy1 = psA.tile([128, N], F32, tag="psA", name="psy_1")
            proj_partial(psy0, 0, range(5), False)
            for kt in range(TOK_TILES):
                emit_attnv_kt(5, 0, kt, pso0)
            raw0 = emit_drain(5, 0, pso0)
            st5_0 = emit_norm_bounce(5, 0, raw0)
            pso1 = psB.tile([128, N], F32, tag="psB", name="pso_5_1")
            proj_partial(psy1, 1, range(5), False)
            for kt in range(TOK_TILES):
                emit_attnv_kt(5, 1, kt, pso1)
            emit_norm_mult(5, 0, st5_0)

            raw1 = aoraw_pool.tile([65, N], F32, tag="aoraw", name="raw_11")
            denb5 = den_pool.tile([64, N], F32, tag="den", name="denb5_1")
            denr5 = den_pool.tile([64, N], F32, tag="den", name="denr5_1")

            def bounce5(qh):
                sl = slice(qh * 512, (qh + 1) * 512)
                dend = dram_pool.tile(
                    [1, 512], F32, tag="dendh", name=f"dendh_1_{qh}"
                )
                nc.sync.dma_start(dend, raw1[64:65, sl])
                dend_bcast = bass.AP(
                    tensor=dend.tensor,
                    offset=dend.offset,
                    ap=[[0, 64]] + list(dend.ap[1:]),
                )
                nc.sync.dma_start(denb5[:, sl], dend_bcast)

            def norm5_mult(sl):
                nc.vector.reciprocal_approx_fast(out=denr5[:, sl], in_=denb5[:, sl])
                nc.vector.tensor_tensor(
                    aoT_sb[64:128, 5, sl], raw1[0:64, sl], denr5[:, sl], MULT
                )

            with tc.high_priority():
                nc.vector.tensor_copy(raw1[:, 0:512], pso1[0:65, 0:512])
            bounce5(0)
            with tc.high_priority():
                nc.vector.tensor_copy(raw1[:, 512:N], pso1[0:65, 512:N])
            bounce5(1)
            # pool B frees as the drains retire: fill the norm latency
            # with proj partials for m=2,3
            psy2 = psB.tile([128, N], F32, tag="psB", name="psy_2")
            proj_partial(psy2, 2, range(5), False)
            psy3 = psB.tile([128, N], F32, tag="psB", name="psy_3")
            proj_partial(psy3, 3, range(5), False)
            norm5_mult(slice(0, 512))

            # ---- output projection (m=0..3 partials already queued) ----
            for psy, m in ((psy0, 0), (psy1, 1), (psy2, 2), (psy3, 3)):
                proj_partial(psy, m, range(5, D_SUB), True)
            norm5_mult(slice(512, N))
            for psy, m in ((psy0, 0), (psy1, 1), (psy2, 2), (psy3, 3)):
                proj_finish(psy, m)
            for m, pool, tg in ((4, psA, "psA"), (5, psA, "psA"), (6, psB, "psB"), (7, psB, "psB")):
                psy = pool.tile([128, N], F32, tag=tg, name=f"psy_{m}")
                proj_partial(psy, m, range(D_SUB), True)
                proj_finish(psy, m)

    nc.compile()
    return nc


def _in_maps(x, w_qkv, b_qkv, w_proj, b_proj):
    w_qkv = np.asarray(w_qkv, dtype=np.float32)
    b_qkv = np.asarray(b_qkv, dtype=np.float32)
    w_proj = np.asarray(w_proj, dtype=np.float32)
    b_proj = np.asarray(b_proj, dtype=np.float32)
    wqk16 = np.ascontiguousarray(w_qkv[:, :F_QK], dtype=np.float16)
    wv16 = np.ascontiguousarray(w_qkv[:, F_QK:], dtype=np.float16)
    wp16 = np.ascontiguousarray(w_proj, dtype=np.float16)
    bqk_col = np.ascontiguousarray(
        b_qkv[:F_QK].reshape(12, 128).T, dtype=np.float32
    )
    bv16 = np.ascontiguousarray(b_qkv[F_QK:], dtype=np.float16)
    bp16 = np.ascontiguousarray(b_proj, dtype=np.float16)
    maps = []
    for c in range(N_CORES):
        maps.append(
            {
                "xt": np.ascontiguousarray(
                    np.asarray(x[c], dtype=np.float32).T.astype(np.float16)
                ),
                "wqk": wqk16,
                "wv": wv16,
                "wproj": wp16,
                "bqk": bqk_col,
                "bv": bv16,
                "bp": bp16,
            }
        )
    return maps


def kernel(x, w_qkv, b_qkv, w_proj, b_proj):
    global _cached_nc
    if _cached_nc is None:
        _cached_nc = _build()
    from concourse.bass_utils import run_bass_kernel_spmd

    res = run_bass_kernel_spmd(
        _cached_nc,
        _in_maps(x, w_qkv, b_qkv, w_proj, b_proj),
        list(range(N_CORES)),
    )
    return np.stack(
        [res.results[c]["y"].astype(np.float32) for c in range(N_CORES)]
    )


if __name__ == "__main__":
    rng = np.random.default_rng(0)
    x = rng.standard_normal((B, N, D), dtype=np.float32)
    w_qkv = rng.standard_normal((D, 3 * D), dtype=np.float32) * D**-0.5
    b_qkv = rng.standard_normal(3 * D).astype(np.float32) * 0.01
    w_proj = rng.standard_normal((D, D), dtype=np.float32) * D**-0.5
    b_proj = rng.standard_normal(D).astype(np.float32) * 0.01
    y = kernel(x, w_qkv, b_qkv, w_proj, b_proj)
    print(y.shape, y.dtype)


# revision 37
# speedup vs baseline: 1.1082x; 1.0091x over previous
"""Multi-head attention (B=8, N=1024, D=768, H=12) on 8 TRN2 NeuronCores.

Sharding: pure data parallel over batch — each core handles one batch
element; weights are replicated. No collectives.

Dense-PE redesign (baseline 296us -> 190us measured; rel err ~8e-4):
  * fp16 operands everywhere on the PE (host-cast x/w_qkv/w_proj):
    same 1 col/cycle as f32r but half the DMA bytes (6.3MB vs 12.6MB).
    (fp8 was measured in numpy: 4.4e-2 rel err — over the 2e-2 budget.)
  * qkv bias folded into the DVE PSUM->SBUF cast (tensor_scalar with a
    per-partition [128,1] bias column), cast per qh-half so the first
    scores/exps start before all of x has landed.
  * PSUM: pool A 2x[128,1024] = scores pss (double-buffered at qh
    granularity: exp of half k never blocks the scores of half k+1) +
    epilogue proj tiles; pool B 2x[128,1024] = v tiles / qk^T psq /
    one attn@v accumulator at a time.
  * attn@v runs one head (i) per pass with a single accumulator, which
    frees the second pool-B slot so the qk^T prefetch for pair r+2
    runs INSIDE the kt loop as PE filler — no round tails, and the
    exp stream on ACT (the ~107us near-critical path) never starves
    across round boundaries.  Scores for pair r spread over both
    passes (2..5 in pass 1; 6..7 plus next round's 0..1 in pass 2).
  * accumulators released by one high-priority DVE copy; softmax den
    normalized via DRAM-bounce broadcast (partition-step-0 reads are
    only legal from DRAM; gpsimd partition_broadcast reads physical
    partition 0 on HW) + DVE reciprocal/multiply, all off-critical.
  * epilogue: pair-5 i=0 norm completes during the i=1 pass; i=1's
    drain/bounce split at q-halves; proj m-tiles pipelined 4-at-a-time
    across both PSUM pools; fp16 y output (host casts back to f32).
  * scheduler control via tc.high_priority() on the scores->exp chain,
    casts, and drains — the Tile rescheduler otherwise defers them
    behind bulk matmuls (it re-orders by its own cost model).
"""

import sys

sys.path.insert(0, "/opt/trn_rl_repo")

import numpy as np

B, N, D, H, HD = 8, 1024, 768, 12, 64
F_QK = 2 * D  # 1536
SCALE = HD**-0.5
TOK_TILES = N // 128  # 8
D_SUB = D // 128  # 6
N_CORES = 8

_cached_nc = None


def _build():
    import concourse.tile as tile
    from concourse import bacc, mybir

    F32 = mybir.dt.float32
    FP16 = mybir.dt.float16
    EXP = mybir.ActivationFunctionType.Exp
    MULT = mybir.AluOpType.mult
    ADD = mybir.AluOpType.add

    nc = bacc.Bacc("TRN2", target_bir_lowering=False, debug=False)

    xt_d = nc.dram_tensor("xt", [D, N], FP16, kind="ExternalInput").ap()
    wqk_d = nc.dram_tensor("wqk", [D, F_QK], FP16, kind="ExternalInput").ap()
    wv_d = nc.dram_tensor("wv", [D, D], FP16, kind="ExternalInput").ap()
    wproj_d = nc.dram_tensor("wproj", [D, D], FP16, kind="ExternalInput").ap()
    bqk_d = nc.dram_tensor("bqk", [128, 12], F32, kind="ExternalInput").ap()
    bv_d = nc.dram_tensor("bv", [D], FP16, kind="ExternalInput").ap()
    bp_d = nc.dram_tensor("bp", [D], FP16, kind="ExternalInput").ap()
    y_d = nc.dram_tensor("y", [N, D], FP16, kind="ExternalOutput").ap()

    with tile.TileContext(nc) as tc:
        with (
            tc.tile_pool(name="singles", bufs=1) as singles,
            tc.tile_pool(name="qkT", bufs=7) as qkT_pool,
            tc.tile_pool(name="wqk", bufs=16) as wqk_pool,
            tc.tile_pool(name="attnT", bufs=12) as attnT_pool,
            tc.tile_pool(name="aoraw", bufs=4) as aoraw_pool,
            tc.tile_pool(name="den", bufs=4) as den_pool,
            tc.tile_pool(name="yout", bufs=3) as y_pool,
            tc.tile_pool(name="psA", bufs=2, space="PSUM") as psA,
            tc.tile_pool(name="psB", bufs=2, space="PSUM") as psB,
            tc.tile_pool(name="dram", bufs=2, space="DRAM") as dram_pool,
        ):
            # ---- resident SBUF tensors ----
            xT_sb = singles.tile([128, D_SUB, N], FP16)  # 12KB/part
            v_sb = singles.tile([128, TOK_TILES, H * 65], FP16)  # 12.2KB/part
            aoT_sb = singles.tile([128, D_SUB, N], FP16)  # 12KB/part
            wproj_sb = singles.tile([128, D_SUB, D], FP16)  # 9KB/part
            wv_sb = singles.tile([128, D_SUB, D], FP16)  # 9KB/part
            bqk_sb = singles.tile([128, 12], F32)
            bv_sb = singles.tile([1, D], FP16)
            bp_sb = singles.tile([1, D], FP16)
            ones1 = singles.tile([1, 512], FP16)
            ones16 = singles.tile([128, 96], FP16)
            ones_f = singles.tile([128, 512], F32)

            # ---- prologue DMAs, interleaved for earliest PE start ----
            # critical chain: wt(f,d) + x half (d, qh0) feed the first
            # qk^T chains; v-weights follow; wproj trails (needed last).
            xt_r = xt_d.rearrange("(o p) n -> p o n", p=128)

            def dma_wt(eng, f, d):
                wt = wqk_pool.tile([128, 128], FP16, tag="wqk", name=f"wt_{f}_{d}")
                eng.dma_start(wt, wqk_d[d * 128 : (d + 1) * 128, f * 128 : f * 128 + 128])
                return wt

            # three issue queues in parallel: sync=x halves + first wqk
            # tiles (critical chain, interleaved so the f0-qh0 matmuls can
            # start on the first arrivals), gpsimd=biases, scalar=bulk.
            # x halves + f0 weight tiles interleaved on sync (the critical
            # chain); f6 weight tiles on gpsimd (tiny, parallel issue); the
            # bulk wv/wproj LAST on sync so their transfers don't steal
            # DMA-engine bandwidth from the x stream.
            wt_tiles = {}
            nc.gpsimd.dma_start(bqk_sb, bqk_d)
            for d in range(D_SUB):
                wt_tiles[(0, d)] = dma_wt(nc.sync, 0, d)
                nc.sync.dma_start(xT_sb[:, d, 0:512], xt_r[:, d, 0:512])
                wt_tiles[(6, d)] = dma_wt(nc.gpsimd, 6, d)
            for d in range(D_SUB):
                nc.sync.dma_start(xT_sb[:, d, 512:N], xt_r[:, d, 512:N])
            nc.sync.dma_start(wv_sb, wv_d.rearrange("(o p) f -> p o f", p=128))
            nc.gpsimd.dma_start(bv_sb, bv_d[None, :])
            nc.sync.dma_start(wproj_sb, wproj_d.rearrange("(o p) f -> p o f", p=128))
            nc.gpsimd.dma_start(bp_sb, bp_d[None, :])

            nc.vector.memset(ones_f, 1.0)
            nc.vector.tensor_copy(ones1, ones_f[0:1, :])
            nc.vector.tensor_copy(ones16, ones_f[:, 0:96])
            # ones columns of [v | 1] slots
            v_ones_view = v_sb.rearrange("p s (h c) -> p s h c", c=65)[:, :, :, 64]
            nc.vector.tensor_copy(
                v_ones_view, ones16.rearrange("p (s h) -> p s h", s=8)
            )

            qk_tiles = {}

            # ---- qk^T: one 128-feature tile (f in 0..11), fp16 out ----
            # bias folded into the PSUM->SBUF cast (per-partition add).
            # matmuls and cast are separately emittable so round tails can
            # order the DVE queue as [drains, casts] — the drains gate the
            # next round's attn@v.
            qk_psq = {}
            qk_cast_left = {}

            def emit_qk_matmuls(f, wt_eng, qhs=(0, 1), pool=None):
                if (f, 0) not in wt_tiles:
                    for d in range(D_SUB):
                        wt_tiles[(f, d)] = dma_wt(wt_eng, f, d)
                if f not in qk_psq:
                    pool = pool or psA
                    tg = "psA" if pool is psA else "psB"
                    qk_psq[f] = pool.tile([128, N], F32, tag=tg, name=f"psq_{f}")
                    qk_cast_left[f] = 2
                psq = qk_psq[f]
                for qh in qhs:
                    sl = slice(qh * 512, (qh + 1) * 512)
                    for d in range(D_SUB):
                        nc.tensor.matmul(
                            psq[:, sl],
                            lhsT=wt_tiles[(f, d)],
                            rhs=xT_sb[:, d, sl],
                            start=(d == 0),
                            stop=(d == D_SUB - 1),
                        )

            def emit_qk_cast(f, qhs=(0, 1)):
                # per-qh-half casts: a half only needs the x halves that fed
                # it, so the first scores/exps can start before all of x has
                # landed
                psq = qk_psq[f]
                if f not in qk_tiles:
                    qk_tiles[f] = qkT_pool.tile(
                        [128, N], FP16, tag="qkT", name=f"qkT_{f}"
                    )
                qt = qk_tiles[f]
                with tc.high_priority():
                    for qh in qhs:
                        sl = slice(qh * 512, (qh + 1) * 512)
                        nc.vector.tensor_scalar(
                            out=qt[:, sl], in0=psq[:, sl],
                            scalar1=bqk_sb[:, f : f + 1], scalar2=None, op0=ADD,
                        )
                        qk_cast_left[f] -= 1
                if qk_cast_left[f] == 0:
                    del qk_psq[f]

            def emit_qk_tile(f, wt_eng, pool=None):
                emit_qk_matmuls(f, wt_eng, pool=pool)
                emit_qk_cast(f)

            # ---- v m-tile: natural layout, scattered into 65-slots ----
            def emit_v_tile(m):
                psv = psB.tile([128, N], F32, tag="psB", name=f"psv_{m}")
                for n0, nsz in ((0, 512), (512, 256)):
                    sl = slice(n0, n0 + nsz)
                    for d in range(D_SUB):
                        nc.tensor.matmul(
                            psv[:, sl],
                            lhsT=xT_sb[:, d, m * 128 : (m + 1) * 128],
                            rhs=wv_sb[:, d, sl],
                            start=(d == 0),
                            stop=False,
                        )
                    nc.tensor.matmul(
                        psv[:, sl],
                        lhsT=ones1[0:1, 0:128],
                        rhs=bv_sb[0:1, sl],
                        start=False,
                        stop=True,
                    )
                nc.vector.tensor_copy(
                    v_sb[:, m, :].rearrange("p (h c) -> p h c", c=65)[:, :, 0:64],
                    psv[:, 0:D].rearrange("p (h c) -> p h c", c=64),
                )

            # ---- attention rounds, software-pipelined over head pairs ----
            attn_tiles = {}  # (pair, kt) -> [128, 2048] fp16: [A0|B0|A1|B1]
            pso_live = {}

            def emit_scores_half(p, kt, qh):
                # the exp stream on ACT is the near-critical path: keep the
                # whole scores->exp chain at the front of the scheduler's
                # priority heap so it is never deferred behind bulk matmuls
                qT = qk_tiles[p]
                kT = qk_tiles[6 + p]
                pss = psA.tile([128, N], F32, tag="psA", name=f"pss_{p}_{kt}_{qh}")
                with tc.high_priority():
                    for i in range(2):
                        pb = slice(64 * i, 64 * i + 64)
                        nc.tensor.matmul(
                            pss[:, i * 512 : i * 512 + 512],
                            lhsT=kT[pb, kt * 128 : (kt + 1) * 128],
                            rhs=qT[pb, qh * 512 : (qh + 1) * 512],
                            start=True,
                            stop=True,
                        )
                    at = attn_tiles[(p, kt)]
                    nc.scalar.activation(
                        at[:, qh * 1024 : (qh + 1) * 1024], pss, func=EXP, scale=SCALE
                    )

            def emit_scores_kt(p, kt, qhs=(0, 1)):
                if (p, kt) not in attn_tiles:
                    attn_tiles[(p, kt)] = attnT_pool.tile(
                        [128, 2 * N], FP16, tag="attnT", name=f"at_{p}_{kt}"
                    )
                for qh in qhs:
                    emit_scores_half(p, kt, qh)

            def emit_attnv_kt(p, i, kt, pso):
                at = attn_tiles[(p, kt)]
                h = 2 * p + i
                for qh in range(2):
                    osl = slice(qh * 512, (qh + 1) * 512)
                    isl = slice(qh * 1024 + i * 512, qh * 1024 + i * 512 + 512)
                    nc.tensor.matmul(
                        pso[0:65, osl],
                        lhsT=v_sb[:, kt, h * 65 : h * 65 + 65],
                        rhs=at[:, isl],
                        start=(kt == 0),
                        stop=(kt == TOK_TILES - 1),
                    )
                if i == 1:
                    del attn_tiles[(p, kt)]

            def emit_drain(p, i, pso):
                # single DVE copy releases the PSUM accumulator fast; high
                # priority so it isn't queued behind other DVE work (it
                # gates the next pass's attn@v accumulator)
                h = 2 * p + i
                raw = aoraw_pool.tile([65, N], F32, tag="aoraw", name=f"raw_{h}")
                with tc.high_priority():
                    nc.vector.tensor_copy(raw, pso[0:65, :])
                return raw

            def emit_norm_bounce(p, i, raw):
                # den row -> DRAM-bounce broadcast to 64 partitions
                # (partition-step-0 read is legal from DRAM; the gpsimd
                # partition_broadcast reads physical partition 0 on HW)
                import concourse.bass as bass

                h = 2 * p + i
                dend = dram_pool.tile([1, N], F32, tag="dend", name=f"dend_{h}")
                nc.sync.dma_start(dend, raw[64:65, :])
                denb = den_pool.tile([64, N], F32, tag="den", name=f"denb_{h}")
                dend_bcast = bass.AP(
                    tensor=dend.tensor,
                    offset=dend.offset,
                    ap=[[0, 64]] + list(dend.ap[1:]),
                )
                nc.sync.dma_start(denb, dend_bcast)
                denr = den_pool.tile([64, N], F32, tag="den", name=f"denr_{h}")
                return (raw, denb, denr)

            def emit_norm_mult(p, i, st, sl=slice(0, N)):
                raw, denb, denr = st
                nc.vector.reciprocal_approx_fast(out=denr[:, sl], in_=denb[:, sl])
                nc.vector.tensor_tensor(
                    aoT_sb[64 * i : 64 * i + 64, p, sl],
                    raw[0:64, sl],
                    denr[:, sl],
                    MULT,
                )

            def emit_norm(p, i, raw):
                emit_norm_mult(p, i, emit_norm_bounce(p, i, raw))

            # proj helpers (epilogue, pairwise m-tiles in pool A)
            def proj_partial(psy, m, d_range, with_bias):
                for n0, nsz in ((0, 512), (512, 256)):
                    sl = slice(n0, n0 + nsz)
                    for d in d_range:
                        nc.tensor.matmul(
                            psy[:, sl],
                            lhsT=aoT_sb[:, d, m * 128 : (m + 1) * 128],
                            rhs=wproj_sb[:, d, sl],
                            start=(d == 0),
                            stop=False,
                        )
                    if with_bias:
                        nc.tensor.matmul(
                            psy[:, sl],
                            lhsT=ones1[0:1, 0:128],
                            rhs=bp_sb[0:1, sl],
                            start=False,
                            stop=True,
                        )

            def proj_finish(psy, m):
                ysb = y_pool.tile([128, D], FP16, tag="ysb", name=f"ysb_{m}")
                nc.vector.tensor_copy(ysb, psy[:, 0:D])
                nc.sync.dma_start(y_d[m * 128 : (m + 1) * 128, :], ysb)

            # ---- prologue: pair-0 qk tiles half-by-half so the first
            # scores/exps start as soon as the qh0 x-halves land (ACT's
            # ~110us exp stream is the near-critical path).  The prologue
            # psq accumulators live in pool B so the scores' pss rotation
            # in pool A is never blocked behind a late cast. ----
            emit_qk_matmuls(0, nc.sync, qhs=(0,), pool=psB)
            emit_qk_matmuls(6, nc.sync, qhs=(0,), pool=psB)
            emit_qk_cast(0, qhs=(0,))
            emit_qk_cast(6, qhs=(0,))
            emit_scores_kt(0, 0, qhs=(0,))
            emit_scores_kt(0, 1, qhs=(0,))
            emit_qk_matmuls(0, nc.sync, qhs=(1,))
            emit_qk_matmuls(6, nc.sync, qhs=(1,))
            emit_qk_cast(0, qhs=(1,))
            emit_qk_cast(6, qhs=(1,))
            emit_scores_kt(0, 0, qhs=(1,))
            emit_scores_kt(0, 1, qhs=(1,))
            emit_qk_tile(1, nc.gpsimd, pool=psB)  # q heads 2,3
            emit_v_tile(0)
            emit_qk_tile(7, nc.gpsimd, pool=psB)  # k heads 2,3
            emit_v_tile(1)

            # round 0: remaining v tiles + scores pair 0 (skewed +2), with
            # the pair-2 qk prefetch interleaved in pool B alongside the v
            # accumulators (one psv + one psq live at a time)
            for j in range(2, TOK_TILES):
                emit_v_tile(j)
                emit_scores_kt(0, j)
                if j == 3:
                    emit_qk_matmuls(2, nc.gpsimd, qhs=(0,), pool=psB)
                elif j == 4:
                    emit_qk_matmuls(2, nc.gpsimd, qhs=(1,))
                    emit_qk_cast(2)
                elif j == 5:
                    emit_qk_matmuls(8, nc.gpsimd, qhs=(0,), pool=psB)
                elif j == 6:
                    emit_qk_matmuls(8, nc.gpsimd, qhs=(1,))
                    emit_qk_cast(8)
            emit_scores_kt(1, 0)
            emit_scores_kt(1, 1)

            # rounds 1..5: attn@v pair r-1 one head (i) at a time — a single
            # PSUM accumulator per pass frees the second pool-B slot for the
            # qk^T prefetch to run INSIDE the loop as PE filler; scores for
            # pair r (and the next round's first two) spread over both
            # passes so the exp stream on ACT never starves.  No round tail.
            for r in range(1, 6):
                qk_a, qk_b = r + 2, 6 + r + 2
                has_qk = qk_a < 6
                pso0 = psB.tile([128, N], F32, tag="psB", name=f"pso_{r - 1}_0")
                for kt in range(TOK_TILES):
                    emit_attnv_kt(r - 1, 0, kt, pso0)
                    if kt % 2 == 0:
                        emit_scores_kt(r, 2 + kt // 2)
                    if has_qk:
                        if kt == 1:
                            emit_qk_matmuls(qk_a, nc.gpsimd, qhs=(0,), pool=psB)
                        elif kt == 3:
                            emit_qk_matmuls(qk_a, nc.gpsimd, qhs=(1,))
                        elif kt == 5:
                            emit_qk_cast(qk_a)
                raw0 = emit_drain(r - 1, 0, pso0)
                pso1 = psB.tile([128, N], F32, tag="psB", name=f"pso_{r - 1}_1")
                emit_norm_bounce_st = emit_norm_bounce(r - 1, 0, raw0)
                for kt in range(TOK_TILES):
                    emit_attnv_kt(r - 1, 1, kt, pso1)
                    if kt == 0:
                        emit_scores_kt(r, 6)
                    elif kt == 2:
                        emit_scores_kt(r, 7)
                    elif kt == 4 and r < 5:
                        emit_scores_kt(r + 1, 0)
                    elif kt == 6 and r < 5:
                        emit_scores_kt(r + 1, 1)
                    if has_qk:
                        if kt == 1:
                            emit_qk_matmuls(qk_b, nc.gpsimd, qhs=(0,), pool=psB)
                        elif kt == 3:
                            emit_qk_matmuls(qk_b, nc.gpsimd, qhs=(1,))
                        elif kt == 5:
                            emit_qk_cast(qk_b)
                emit_norm_mult(r - 1, 0, emit_norm_bounce_st)
                raw1 = emit_drain(r - 1, 1, pso1)
                emit_norm(r - 1, 1, raw1)

            # round 6: attn@v pair 5, two passes, proj pipelined in.  The
            # i=0 head's norm completes during the i=1 pass; i=1's drain +
            # den bounce are split at q-halves so the projection's d5
            # chunks for m=0..3 unblock right after the h0 mult.
            import concourse.bass as bass

            pso0 = psB.tile([128, N], F32, tag="psB", name="pso_5_0")
            psy0 = psA.tile([128, N], F32, tag="psA", name="psy_0")
            psy1 = psA.tile([128, N], F32, tag="psA", name="psy_1")
            proj_partial(psy0, 0, range(5), False)
            for kt in range(TOK_TILES):
                emit_attnv_kt(5, 0, kt, pso0)
            raw0 = emit_drain(5, 0, pso0)
            st5_0 = emit_norm_bounce(5, 0, raw0)
            pso1 = psB.tile([128, N], F32, tag="psB", name="pso_5_1")
            proj_partial(psy1, 1, range(5), False)
            for kt in range(TOK_TILES):
                emit_attnv_kt(5, 1, kt, pso1)
            emit_norm_mult(5, 0, st5_0)

            raw1 = aoraw_pool.tile([65, N], F32, tag="aoraw", name="raw_11")
            denb5 = den_pool.tile([64, N], F32, tag="den", name="denb5_1")
            denr5 = den_pool.tile([64, N], F32, tag="den", name="denr5_1")

            def bounce5(qh):
                sl = slice(qh * 512, (qh + 1) * 512)
                dend = dram_pool.tile(
                    [1, 512], F32, tag="dendh", name=f"dendh_1_{qh}"
                )
                nc.sync.dma_start(dend, raw1[64:65, sl])
                dend_bcast = bass.AP(
                    tensor=dend.tensor,
                    offset=dend.offset,
                    ap=[[0, 64]] + list(dend.ap[1:]),
                )
                nc.sync.dma_start(denb5[:, sl], dend_bcast)

            def norm5_mult(sl):
                nc.vector.reciprocal_approx_fast(out=denr5[:, sl], in_=denb5[:, sl])
                nc.vector.tensor_tensor(
                    aoT_sb[64:128, 5, sl], raw1[0:64, sl], denr5[:, sl], MULT
                )

            with tc.high_priority():
                nc.vector.tensor_copy(raw1[:, 0:512], pso1[0:65, 0:512])
            bounce5(0)
            with tc.high_priority():
                nc.vector.tensor_copy(raw1[:, 512:N], pso1[0:65, 512:N])
            bounce5(1)
            # pool B frees as the drains retire: fill the norm latency
            # with proj partials for m=2,3
            psy2 = psB.tile([128, N], F32, tag="psB", name="psy_2")
            proj_partial(psy2, 2, range(5), False)
            psy3 = psB.tile([128, N], F32, tag="psB", name="psy_3")
            proj_partial(psy3, 3, range(5), False)
            norm5_mult(slice(0, 512))

            # ---- output projection (m=0..3 partials already queued) ----
            for psy, m in ((psy0, 0), (psy1, 1), (psy2, 2), (psy3, 3)):
                proj_partial(psy, m, range(5, D_SUB), True)
            norm5_mult(slice(512, N))
            for psy, m in ((psy0, 0), (psy1, 1), (psy2, 2), (psy3, 3)):
                proj_finish(psy, m)
            for m, pool, tg in ((4, psA, "psA"), (5, psA, "psA"), (6, psB, "psB"), (7, psB, "psB")):
                psy = pool.tile([128, N], F32, tag=tg, name=f"psy_{m}")
                proj_partial(psy, m, range(D_SUB), True)
                proj_finish(psy, m)

    nc.compile()
    return nc


def _in_maps(x, w_qkv, b_qkv, w_proj, b_proj):
    w_qkv = np.asarray(w_qkv, dtype=np.float32)
    b_qkv = np.asarray(b_qkv, dtype=np.float32)
    w_proj = np.asarray(w_proj, dtype=np.float32)
    b_proj = np.asarray(b_proj, dtype=np.float32)
    wqk16 = np.ascontiguousarray(w_qkv[:, :F_QK], dtype=np.float16)
    wv16 = np.ascontiguousarray(w_qkv[:, F_QK:], dtype=np.float16)
    wp16 = np.ascontiguousarray(w_proj, dtype=np.float16)
    bqk_col = np.ascontiguousarray(
        b_qkv[:F_QK].reshape(12, 128).T, dtype=np.float32
    )
    bv16 = np.ascontiguousarray(b_qkv[F_QK:], dtype=np.float16)
    bp16 = np.ascontiguousarray(b_proj, dtype=np.float16)
    maps = []
    for c in range(N_CORES):
        maps.append(
            {
                "xt": np.ascontiguousarray(
                    np.asarray(x[c], dtype=np.float32).T.astype(np.float16)
                ),
                "wqk": wqk16,
                "wv": wv16,
                "wproj": wp16,
                "bqk": bqk_col,
                "bv": bv16,
                "bp": bp16,
            }
        )
    return maps


def kernel(x, w_qkv, b_qkv, w_proj, b_proj):
    global _cached_nc
    if _cached_nc is None:
        _cached_nc = _build()
    from concourse.bass_utils import run_bass_kernel_spmd

    res = run_bass_kernel_spmd(
        _cached_nc,
        _in_maps(x, w_qkv, b_qkv, w_proj, b_proj),
        list(range(N_CORES)),
    )
    return np.stack(
        [res.results[c]["y"].astype(np.float32) for c in range(N_CORES)]
    )


if __name__ == "__main__":
    rng = np.random.default_rng(0)
    x = rng.standard_normal((B, N, D), dtype=np.float32)
    w_qkv = rng.standard_normal((D, 3 * D), dtype=np.float32) * D**-0.5
    b_qkv = rng.standard_normal(3 * D).astype(np.float32) * 0.01
    w_proj = rng.standard_normal((D, D), dtype=np.float32) * D**-0.5
    b_proj = rng.standard_normal(D).astype(np.float32) * 0.01
    y = kernel(x, w_qkv, b_qkv, w_proj, b_proj)
    print(y.shape, y.dtype)
